# revision 11
# baseline (speedup 1.0000x reference)
"""GAT Trainium kernel v3: 256B gather rows + PSUM-decontended DVE ops.

v3 changes vs v2 (driven by NTFF profile: GpSimd 86% busy, 3.65ms in
DMAGatherAnt at ~7.8ns/idx while SDMA queues were only ~20% busy):
- The Q7 descriptor generator shares an SBUF port pair with DVE; every
  2-input DVE op (is_equal, multiply, leaky) locked GpSimd out for its whole
  duration. All hot DVE ops are now 1-SBUF-port: one operand lives in PSUM
  (iota for gen_S, exp/scores in PSUM), leaky-relu + exp moved to the ACT
  engine (Lrelu/Exp), memset of the accumulator moved to ACT.
- G rows shrunk 512B -> 256B (128 bf16 = h only). el is no longer carried in
  the row; the consumer computes el = sum(h*al) per head with a DVE mult +
  tensor_reduce per batch. Halves gather bytes and AllGather wire volume.
- Layer-3 rows stay 256B: [h x40 | el x1 | pad] bf16.
"""
from contextlib import ExitStack
import numpy as np
import ml_dtypes
import concourse.bass as bass
import concourse.tile as tile
from concourse import bacc, mybir
from concourse.masks import make_identity

F32 = mybir.dt.float32
BF16 = mybir.dt.bfloat16
I16 = mybir.dt.int16
BF = ml_dtypes.bfloat16

IN, HID, HEADS, OUT = 256, 32, 4, 40
HH = HID * HEADS  # 128
NEG_SLOPE = 0.2
BT = 16  # tiles per gather batch (16*128 = 2048 idx/call)


def wrap16(a):
    n = a.shape[0]
    assert n % 16 == 0
    blk = a.reshape(-1, 16).T  # [16, n/16]
    return np.tile(blk, (8, 1)).astype(np.int16)


def host_preprocess(src, dst, n_nodes, n_cores=8, n_chunks=4):
    NL = n_nodes // n_cores
    assert NL * n_cores == n_nodes
    NB = (NL + 127) // 128
    CS = (n_nodes + n_chunks - 1) // n_chunks
    assert CS <= 32767

    QR = NL // n_chunks
    assert QR * n_chunks == NL
    src = np.asarray(src); dst = np.asarray(dst)
    core_of = dst // NL
    per_core = []
    counts = np.zeros((n_cores, NB, n_chunks), np.int64)
    for c in range(n_cores):
        m = core_of == c
        s, d = src[m], dst[m]
        dloc = d - c * NL
        # chunk q holds quarter q of every core's slab: node (cs, i) lives at
        # chunk i//QR, row cs*QR + i%QR  (so one quarter-slab AllGather
        # completes one whole chunk)
        cs_ = s // NL
        i_ = s % NL
        ch = i_ // QR
        sloc = cs_ * QR + (i_ % QR)
        order = np.lexsort((dloc, ch))
        dloc, ch, sloc = dloc[order], ch[order], sloc[order]
        b = dloc // 128
        per_core.append((sloc, dloc, ch, b))
        for chh in range(n_chunks):
            mm = ch == chh
            bb, cnt = np.unique(b[mm], return_counts=True)
            counts[c, bb, chh] = cnt
    T = np.ceil(counts.max(axis=0) / 128).astype(np.int64)  # [NB, n_chunks]

    segs = []  # chunk-major: (chunk, block, tiles)
    for chh in range(n_chunks):
        for b in range(NB):
            if T[b, chh] > 0:
                segs.append((chh, b, int(T[b, chh])))
    n_tiles = sum(t for _, _, t in segs)
    total_slots = n_tiles * 128

    batches = []
    cur = None
    tglob = 0
    for chh, b, t in segs:
        for _ in range(t):
            if cur is None or cur["chunk"] != chh or cur["nt"] >= BT:
                if cur is not None:
                    batches.append(cur)
                cur = {"chunk": chh, "t0": tglob, "nt": 0}
            cur["nt"] += 1
            tglob += 1
    if cur is not None:
        batches.append(cur)
    assert tglob == n_tiles

    core_data = []
    for c in range(n_cores):
        sloc, dloc, ch, b = per_core[c]
        src16 = np.zeros(total_slots, np.int16)
        dcol = np.full((128, n_tiles), -1.0, np.float32)  # pad never matches
        S2 = np.zeros((128, total_slots), BF)  # [d, t*128+e] one-hot
        pos = 0
        for chh, bb, t in segs:
            m = (ch == chh) & (b == bb)
            idx = np.nonzero(m)[0]
            n = len(idx)
            cap = t * 128
            assert n <= cap, (c, chh, bb, n, cap)
            sl = sloc[idx]; dl = dloc[idx]
            src16[pos:pos + n] = sl
            e_in_seg = np.arange(n)
            tt = pos // 128 + e_in_seg // 128
            ee = e_in_seg % 128
            dloc_in_b = dl - bb * 128
            dcol[ee, tt] = dloc_in_b
            S2[dloc_in_b, tt * 128 + ee] = 1.0
            pos += cap
        assert pos == total_slots
        core_data.append(dict(
            src16=wrap16(src16),
            dcol=dcol,
            S2=S2,
        ))

    return dict(
        n_cores=n_cores, n_nodes=n_nodes, NL=NL, NB=NB, CS=CS, QR=QR,
        n_chunks=n_chunks, segs=segs, batches=batches, n_tiles=n_tiles,
        total_slots=total_slots, core_data=core_data,
    )


def host_weights(W1, al1, ar1, b1, W2, al2, ar2, b2, W3, al3, ar3, b3):
    def bd(al):
        al = np.asarray(al, np.float32)
        H, F = al.shape
        out = np.zeros((H * F, H), np.float32)
        for h in range(H):
            out[h * F:(h + 1) * F, h] = al[h]
        return out
    W1 = np.asarray(W1, np.float32); W2 = np.asarray(W2, np.float32); W3 = np.asarray(W3, np.float32)
    # producer rows carry only h (+er table); el computed on the consumer
    W1ext = np.concatenate([W1, W1 @ bd(ar1)], axis=1)            # [IN, 132]
    W2ext = np.concatenate([W2, W2 @ bd(ar2)], axis=1)            # [HH, 132]
    W3ext = np.concatenate([W3, W3 @ bd(al3), W3 @ bd(ar3)], axis=1)  # [HH, 42]
    b1rep = np.tile(np.asarray(b1, np.float32).reshape(1, HH), (128, 1))
    b2rep = np.tile(np.asarray(b2, np.float32).reshape(1, HH), (128, 1))
    b3rep = np.tile(np.asarray(b3, np.float32).reshape(1, OUT), (128, 1))
    iota = np.tile(np.arange(128, dtype=np.float32)[None, :], (128, 1))
    al1rep = np.tile(np.asarray(al1, np.float32).reshape(1, HH), (128, 1))
    al2rep = np.tile(np.asarray(al2, np.float32).reshape(1, HH), (128, 1))
    return dict(W1ext=W1ext.astype(BF), W2ext=W2ext.astype(BF),
                W3ext=W3ext.astype(BF),
                b1rep=b1rep, b2rep=b2rep, b3rep=b3rep,
                iota=iota,
                al1rep=al1rep.astype(BF), al2rep=al2rep.astype(BF))


def apx(base_ap, col_off, dims):
    """AP at column offset of a [128, W] tile with custom free dims."""
    b = base_ap[:, col_off:col_off + 1]
    return bass.AP(b.tensor, b.offset, [b.ap[0]] + [list(d) for d in dims])


def build_program(plan, stage=99):
    C = plan["n_cores"]; NL = plan["NL"]; NB = plan["NB"]
    NT = plan["n_tiles"]; TS = plan["total_slots"]
    NLP = NB * 128

    nc = bacc.Bacc("TRN2", target_bir_lowering=False, debug=False, num_devices=C)

    featT = nc.dram_tensor("featT", [IN, NL], BF16, kind="ExternalInput").ap()
    W1e = nc.dram_tensor("W1ext", [IN, 132], BF16, kind="ExternalInput").ap()
    W2e = nc.dram_tensor("W2ext", [HH, 132], BF16, kind="ExternalInput").ap()
    W3e = nc.dram_tensor("W3ext", [HH, 42], BF16, kind="ExternalInput").ap()
    B1 = nc.dram_tensor("b1rep", [128, HH], F32, kind="ExternalInput").ap()
    B2 = nc.dram_tensor("b2rep", [128, HH], F32, kind="ExternalInput").ap()
    B3 = nc.dram_tensor("b3rep", [128, OUT], F32, kind="ExternalInput").ap()
    AL1 = nc.dram_tensor("al1rep", [128, HH], BF16, kind="ExternalInput").ap()
    AL2 = nc.dram_tensor("al2rep", [128, HH], BF16, kind="ExternalInput").ap()
    SRC = nc.dram_tensor("src16", [128, TS // 16], I16, kind="ExternalInput").ap()
    DCOL = nc.dram_tensor("dcol", [128, NT], F32, kind="ExternalInput").ap()
    IOTA = nc.dram_tensor("iota", [128, 128], F32, kind="ExternalInput").ap()
    S2D = nc.dram_tensor("S2", [128, TS], BF16, kind="ExternalInput").ap()
    OUTT = nc.dram_tensor("out", [NLP, OUT], F32, kind="ExternalOutput").ap()

    G1s = nc.dram_tensor("G1slab", [NL, 128], BF16).ap()
    G2s = nc.dram_tensor("G2slab", [NL, 128], BF16).ap()
    G3s = nc.dram_tensor("G3slab", [NL, 128], BF16).ap()
    G1 = nc.dram_tensor("G1", [C * NL, 128], BF16, addr_space="Shared").ap()
    G2 = nc.dram_tensor("G2", [C * NL, 128], BF16, addr_space="Shared").ap()
    G3 = nc.dram_tensor("G3", [C * NL, 128], BF16, addr_space="Shared").ap()

    rg = [list(range(C))]

    QR = plan["QR"]; NQ = plan["n_chunks"]; CSz = plan["CS"]

    def allgather(slab, full):
        # quarter-interleaved: AG of slab rows [q*QR,(q+1)*QR) from all cores
        # completes gather-chunk q of `full` -> edge phase chunk q can start
        # while later quarters are still gathering
        for q in range(NQ):
            nc.gpsimd.collective_compute(
                "AllGather", mybir.AluOpType.bypass,
                replica_groups=rg,
                ins=[slab[q * QR:(q + 1) * QR, :]],
                outs=[full[q * CSz:(q + 1) * CSz, :]])

    with tile.TileContext(nc) as tc, ExitStack() as ctx:
        const = ctx.enter_context(tc.tile_pool(name="const", bufs=1))
        accp = ctx.enter_context(tc.tile_pool(name="acc", bufs=1))
        constp = ctx.enter_context(tc.tile_pool(name="constp", bufs=1, space="PSUM"))

        w1sb = const.tile([128, 2 * 132], BF16)
        nc.sync.dma_start(w1sb[:, 0:132], W1e[0:128, :])
        nc.sync.dma_start(w1sb[:, 132:264], W1e[128:256, :])
        w2sb = const.tile([128, 132], BF16)
        nc.sync.dma_start(w2sb[:], W2e[:, :])
        w3sb = const.tile([128, 42], BF16)
        nc.sync.dma_start(w3sb[:], W3e[:, :])
        b1sb = const.tile([128, HH], F32)
        nc.sync.dma_start(b1sb[:], B1[:, :])
        b2sb = const.tile([128, HH], F32)
        nc.sync.dma_start(b2sb[:], B2[:, :])
        b3sb = const.tile([128, OUT], F32)
        nc.sync.dma_start(b3sb[:], B3[:, :])
        al1sb = const.tile([128, HH], BF16)
        nc.sync.dma_start(al1sb[:], AL1[:, :])
        al2sb = const.tile([128, HH], BF16)
        nc.sync.dma_start(al2sb[:], AL2[:, :])
        ident = const.tile([128, 128], F32)
        make_identity(nc, ident[:])
        iotasb = const.tile([128, 128], F32)
        nc.sync.dma_start(iotasb[:], IOTA[:, :])
        dcolsb = const.tile([128, NT], F32)
        nc.sync.dma_start(dcolsb[:], DCOL[:, :])
        srcsb = const.tile([128, TS // 16], I16)
        nc.sync.dma_start(srcsb[:], SRC[:, :])
        # iota parked in PSUM so gen_S is a 1-SBUF-port DVE op
        iotaP = constp.tile([128, 128], F32)
        nc.vector.tensor_copy(iotaP[:], iotasb[:])
        # SBUF er tables, one per layer: [128 (node-in-block), NB*heads]
        er1sb = const.tile([128, NB * 4], BF16)
        er2sb = const.tile([128, NB * 4], BF16)
        er3sb = const.tile([128, NB], BF16)

        def write_node_rows(lp, ps, nr, b, r0, r1, Gn_s, ersb, n_h, n_el):
            """ps [nr, n_h+n_el+n_er] f32 PSUM -> G slab row bf16 + er table."""
            gsb = lp.tile([128, 128], BF16, tag="gsb")
            if n_el:  # layer-3 rows carry el packed after h
                nc.vector.tensor_copy(gsb[:nr, 0:n_h + n_el], ps[:nr, 0:n_h + n_el])
            else:
                nc.vector.tensor_copy(gsb[:nr, 0:n_h], ps[:nr, 0:n_h])
            nc.sync.dma_start(Gn_s[r0:r1, 0:n_h + n_el], gsb[:nr, 0:n_h + n_el])
            ner = ersb.shape[1] // NB
            nc.vector.tensor_copy(ersb[:nr, b * ner:(b + 1) * ner],
                                  ps[:nr, n_h + n_el:n_h + n_el + ner])

        # Layer 1 node phase
        with tc.tile_pool(name="l1n", bufs=3) as lp, \
             tc.tile_pool(name="l1np", bufs=2, space="PSUM") as pp:
            for b in range(NB):
                r0 = b * 128
                r1 = min(r0 + 128, NL)
                nr = r1 - r0
                xt = lp.tile([128, 256], BF16, tag="xt")
                nc.sync.dma_start(xt[:, 0:nr], featT[0:128, r0:r1])
                nc.sync.dma_start(xt[:, 128:128 + nr], featT[128:256, r0:r1])
                ps = pp.tile([128, 132], F32, tag="ps")
                nc.tensor.matmul(ps[:nr, :], xt[:, 0:nr], w1sb[:, 0:132],
                                 start=True, stop=False)
                nc.tensor.matmul(ps[:nr, :], xt[:, 128:128 + nr], w1sb[:, 132:264],
                                 start=False, stop=True)
                write_node_rows(lp, ps, nr, b, r0, r1, G1s, er1sb, 128, 0)

        if stage >= 2:
            allgather(G1s, G1)
        if stage >= 3:
            edge_layer(tc, plan, 1, G1, er1sb, srcsb, dcolsb, iotaP, S2D, accp,
                       w_next=w2sb, b_rep=b1sb, Gn_s=G2s, ersb_n=er2sb,
                       ident=ident, write_node=write_node_rows, alsb=al1sb,
                       node_phase=(stage >= 4))
        if stage >= 5:
            allgather(G2s, G2)
            edge_layer(tc, plan, 2, G2, er2sb, srcsb, dcolsb, iotaP, S2D, accp,
                       w_next=w3sb, b_rep=b2sb, Gn_s=G3s, ersb_n=er3sb,
                       ident=ident, write_node=write_node_rows, alsb=al2sb)
            allgather(G3s, G3)
        if stage >= 6:
            edge_layer3(tc, plan, G3, er3sb, srcsb, dcolsb, iotaP, S2D, accp,
                        b3sb, ident, OUTT)

    nc.compile()
    return nc


def seg_maps(plan):
    seg_of_tile = []
    tile_in_seg = []
    seg_idx_of_tile = []
    last_seg_of_block = {}
    for si, (chh, b, t) in enumerate(plan["segs"]):
        seg_of_tile += [(chh, b, t)] * t
        tile_in_seg += list(range(t))
        seg_idx_of_tile += [si] * t
        last_seg_of_block[b] = si
    return seg_of_tile, tile_in_seg, seg_idx_of_tile, last_seg_of_block


def gen_S(nc, Ssb, dcolsb, iotaP, t0, nt):
    """S[e, k, d] = (dcol[e, t0+k] == iota[d]), bf16.

    dcol read from SBUF, iota from PSUM -> single SBUF read port, so the op
    does not lock GpSimd out of the shared port pair during desc-gen.
    """
    Sv = apx(Ssb[:], 0, [[128, nt], [1, 128]])
    dc = apx(dcolsb[:], t0, [[1, nt], [0, 128]])
    io = apx(iotaP[:], 0, [[0, nt], [1, 128]])
    nc.vector.tensor_tensor(out=Sv, in0=dc, in1=io,
                            op=mybir.AluOpType.is_equal)


def edge_layer(tc, plan, lnum, G, ersb, srcsb, dcolsb, iotaP, S2D, accp,
               w_next, b_rep, Gn_s, ersb_n, ident, write_node, alsb,
               node_phase=True):
    nc = tc.nc
    NL = plan["NL"]; NB = plan["NB"]; CS = plan["CS"]
    seg_of_tile, tile_in_seg, seg_idx_of_tile, last_seg = seg_maps(plan)

    A = accp.tile([128, NB * 132], F32, tag="A")
    nc.scalar.memzero(A[:])

    NOUT = w_next.shape[1]
    n_h = {132: 128, 42: 40}[NOUT]

    with tc.tile_pool(name=f"e{lnum}", bufs=4) as ep, \
         tc.tile_pool(name=f"e{lnum}b", bufs=3) as bp, \
         tc.tile_pool(name=f"e{lnum}s", bufs=4) as sp, \
         tc.tile_pool(name=f"n{lnum}", bufs=3) as np_, \
         tc.tile_pool(name=f"e{lnum}p", bufs=2, space="PSUM") as pp, \
         tc.tile_pool(name=f"e{lnum}e", bufs=2, space="PSUM") as pe, \
         tc.tile_pool(name=f"n{lnum}p", bufs=1, space="PSUM") as npp:

        def node_block(b):
            if not node_phase:
                return
            r0 = b * 128
            r1 = min(r0 + 128, NL)
            nr = r1 - r0
            Ab = A[:, b * 132:(b + 1) * 132]
            # nodeA bank: [0:128) hv, [128:132) rs
            nodeA = npp.tile([128, 132], F32, tag="nodeA")
            rs = nodeA[:, 128:132]
            hv = nodeA[:, 0:128]
            nc.vector.tensor_scalar_max(rs, Ab[:, 128:132], 1e-30)
            nc.vector.reciprocal(rs, rs)
            hvv = hv.rearrange("p (g f) -> p g f", g=4)
            rsb = apx(nodeA[:], 128, [[1, 4], [0, 32]])
            av = Ab[:, 0:128].rearrange("p (g f) -> p g f", g=4)
            nc.vector.tensor_tensor(out=hvv, in0=av, in1=rsb,
                                    op=mybir.AluOpType.mult)
            hp = np_.tile([128, 128], F32, tag="hp")
            nc.vector.tensor_tensor(out=hp[:], in0=hv, in1=b_rep[:],
                                    op=mybir.AluOpType.add)
            nc.scalar.activation(hp[:], hp[:], mybir.ActivationFunctionType.Relu)
            pst = npp.tile([128, 128], F32, tag="pst")
            nc.tensor.transpose(out=pst[:], in_=hp[:], identity=ident[:])
            hpt = np_.tile([128, 128], BF16, tag="hpt")
            nc.vector.tensor_copy(hpt[:], pst[:])
            ps2 = npp.tile([128, NOUT], F32, tag="ps2")
            nc.tensor.matmul(ps2[:nr, :], hpt[:, 0:nr], w_next[:],
                             start=True, stop=True)
            n_el_next = 1 if NOUT == 42 else 0
            n_h_next = 40 if NOUT == 42 else 128
            write_node(np_, ps2, nr, b, r0, r1, Gn_s, ersb_n, n_h_next, n_el_next)

        ps_cur = {}
        for bt in plan["batches"]:
            nt = bt["nt"]; t0 = bt["t0"]; chh = bt["chunk"]
            nidx = nt * 128
            T = ep.tile([128, BT * 128], BF16, tag="T")
            Tv = T[:].rearrange("p (k d) -> p k d", d=128)[:, 0:nt, :]
            nc.gpsimd.dma_gather(
                Tv, G[chh * CS:(chh + 1) * CS, :],
                srcsb[:, t0 * 8:(t0 + nt) * 8],
                nidx, nidx, 128, single_packet=False)
            # S2 one-hot [d, e] stream for er matmuls
            S2sb = sp.tile([128, BT * 128], BF16, tag="S2")
            nc.sync.dma_start(S2sb[:, 0:nt * 128],
                              S2D[:, t0 * 128:(t0 + nt) * 128])
            # escore bank: [0:4BT) erps, [4BT:8BT) sc, [8BT:12BT) leaky, [12BT:16BT) exp
            esc = pe.tile([128, BT * 16], F32, tag="esc")
            erps = esc[:, 0:BT * 4]
            sc = esc[:, BT * 4:BT * 8]
            scl = esc[:, BT * 8:BT * 12]
            excol = esc[:, BT * 12:BT * 16]
            # er[e, (k,h)] = S2_tile.T @ erb_block (lands in score layout)
            for k in range(nt):
                _, b, _ = seg_of_tile[t0 + k]
                nc.tensor.matmul(
                    erps[:, k * 4:(k + 1) * 4],
                    S2sb[:, k * 128:(k + 1) * 128],
                    ersb[:, b * 4:(b + 1) * 4],
                    start=True, stop=True)
            # el[e, (k,h)] = sum_f T[e,(k,h,f)] * al[(h,f)] : mult + reduce
            eltmp = ep.tile([128, BT * 128], BF16, tag="eltmp")
            etv = eltmp[:].rearrange("p (k d) -> p k d", d=128)[:, 0:nt, :]
            alv = apx(alsb[:], 0, [[0, nt], [1, 128]])
            Tvb = T[:].rearrange("p (k d) -> p k d", d=128)[:, 0:nt, :]
            nc.vector.tensor_tensor(out=etv, in0=Tvb, in1=alv,
                                    op=mybir.AluOpType.mult)
            elsb = np_.tile([128, BT * 4], F32, tag="elsb")
            elv3 = eltmp[:].rearrange("p (k g f) -> p (k g) f", g=4, f=32)[:, 0:nt * 4, :]
            nc.vector.tensor_reduce(elsb[:, 0:nt * 4], elv3,
                                    axis=mybir.AxisListType.X,
                                    op=mybir.AluOpType.add)
            # score = leaky(el + er); el SBUF + er PSUM -> sc PSUM (1-port)
            nc.vector.tensor_tensor(out=sc[:, 0:nt * 4], in0=elsb[:, 0:nt * 4],
                                    in1=erps[:, 0:nt * 4],
                                    op=mybir.AluOpType.add)
            # leaky = max(0.2*sc, sc): two 1-PSUM-read DVE ops (DVE may read
            # only one PSUM input per instruction)
            scm = np_.tile([128, BT * 4], F32, tag="scm")
            nc.vector.tensor_scalar_mul(scm[:, 0:nt * 4], sc[:, 0:nt * 4],
                                        NEG_SLOPE)
            nc.vector.tensor_tensor(out=scl[:, 0:nt * 4], in0=scm[:, 0:nt * 4],
                                    in1=sc[:, 0:nt * 4],
                                    op=mybir.AluOpType.max)
            nc.scalar.activation(excol[:, 0:nt * 4], scl[:, 0:nt * 4],
                                 mybir.ActivationFunctionType.Exp)
            B = bp.tile([128, BT * 132], BF16, tag="B")
            Bv = B[:].rearrange("p (k d) -> p k d", d=132)[:, 0:nt, :]
            # alpha into B's payload cols (for the denominator row of ps)
            nc.vector.tensor_copy(Bv[:, :, 128:132],
                                  excol[:].rearrange("p (k d) -> p k d", d=4)[:, 0:nt, :])
            # weighted h: T (SBUF) * exp (PSUM broadcast) -> B (1 SBUF read)
            hw = apx(B[:], 0, [[132, nt], [32, 4], [1, 32]])
            hi = apx(T[:], 0, [[128, nt], [32, 4], [1, 32]])
            ex4 = apx(esc[:], BT * 12, [[4, nt], [1, 4], [0, 32]])
            nc.vector.tensor_tensor(out=hw, in0=hi, in1=ex4,
                                    op=mybir.AluOpType.mult)
            Ssb = sp.tile([128, BT * 128], BF16, tag="S")
            gen_S(nc, Ssb, dcolsb, iotaP, t0, nt)
            for k in range(nt):
                tg = t0 + k
                _, b, tseg = seg_of_tile[tg]
                tin = tile_in_seg[tg]
                if tin == 0:
                    ps_cur[b] = pp.tile([128, 132], F32, tag="ps", name="ps")
                ps = ps_cur[b]
                nc.tensor.matmul(
                    ps[:], Ssb[:, k * 128:(k + 1) * 128],
                    B[:, k * 132:(k + 1) * 132],
                    start=(tin == 0), stop=(tin == tseg - 1))
                if tin == tseg - 1:
                    nc.vector.tensor_tensor(
                        out=A[:, b * 132:(b + 1) * 132],
                        in0=A[:, b * 132:(b + 1) * 132],
                        in1=ps[:], op=mybir.AluOpType.add)
                    del ps_cur[b]
                    if seg_idx_of_tile[tg] == last_seg[b]:
                        node_block(b)
        assert not ps_cur
        for b in range(NB):
            if b not in last_seg:
                node_block(b)


def edge_layer3(tc, plan, G, ersb, srcsb, dcolsb, iotaP, S2D, accp,
                b3sb, ident, OUTT):
    nc = tc.nc
    NL = plan["NL"]; NB = plan["NB"]; CS = plan["CS"]
    seg_of_tile, tile_in_seg, seg_idx_of_tile, last_seg = seg_maps(plan)

    A = accp.tile([128, NB * 132], F32, tag="A")
    Av = A[:, 0:NB * 41]
    nc.scalar.memzero(A[:])

    with tc.tile_pool(name="n3", bufs=1) as no_, \
         tc.tile_pool(name="n3p", bufs=2, space="PSUM") as nop:
        O = no_.tile([128, NB * 40], F32, tag="O")

        def node_block3(b):
            Ab = Av[:, b * 41:(b + 1) * 41]
            rs = nop.tile([128, 4], F32, tag="rs3")
            nc.vector.tensor_scalar_max(rs[:, 0:1], Ab[:, 40:41], 1e-30)
            nc.vector.reciprocal(rs[:, 0:1], rs[:, 0:1])
            rsb = apx(rs[:], 0, [[0, 40]])
            Ob = O[:, b * 40:(b + 1) * 40]
            nc.vector.tensor_tensor(out=Ob, in0=Ab[:, 0:40], in1=rsb,
                                    op=mybir.AluOpType.mult)
            nc.vector.tensor_tensor(out=Ob, in0=Ob, in1=b3sb[:],
                                    op=mybir.AluOpType.add)

        with tc.tile_pool(name="e3", bufs=4) as ep, \
             tc.tile_pool(name="e3b", bufs=3) as bp, \
             tc.tile_pool(name="e3s", bufs=4) as sp, \
             tc.tile_pool(name="e3p", bufs=3, space="PSUM") as pp, \
             tc.tile_pool(name="e3e", bufs=2, space="PSUM") as pe:
            ps_cur = {}
            for bt in plan["batches"]:
                nt = bt["nt"]; t0 = bt["t0"]; chh = bt["chunk"]
                nidx = nt * 128
                T = ep.tile([128, BT * 128], BF16, tag="T3")
                Tv = T[:].rearrange("p (k d) -> p k d", d=128)[:, 0:nt, :]
                nc.gpsimd.dma_gather(
                    Tv, G[chh * CS:(chh + 1) * CS, :],
                    srcsb[:, t0 * 8:(t0 + nt) * 8],
                    nidx, nidx, 128, single_packet=False)
                S2sb = sp.tile([128, BT * 128], BF16, tag="S23")
                nc.sync.dma_start(S2sb[:, 0:nt * 128],
                                  S2D[:, t0 * 128:(t0 + nt) * 128])
                # escore bank: [0:BT) erps, [BT:2BT) sc, [2BT:3BT) leaky, [3BT:4BT) exp
                esc = pe.tile([128, BT * 4], F32, tag="esc3")
                erps = esc[:, 0:BT]
                sc = esc[:, BT:2 * BT]
                scl = esc[:, 2 * BT:3 * BT]
                excol = esc[:, 3 * BT:4 * BT]
                for k in range(nt):
                    _, b, _ = seg_of_tile[t0 + k]
                    nc.tensor.matmul(
                        erps[:, k:k + 1],
                        S2sb[:, k * 128:(k + 1) * 128],
                        ersb[:, b:b + 1],
                        start=True, stop=True)
                # el is packed bf16 in the row at col 40
                scv = sc[:].rearrange("p (k d) -> p k d", d=1)[:, 0:nt, :]
                erv = erps[:].rearrange("p (k d) -> p k d", d=1)[:, 0:nt, :]
                nc.vector.tensor_tensor(out=scv, in0=Tv[:, :, 40:41], in1=erv,
                                        op=mybir.AluOpType.add)
                scm = no_.tile([128, BT], F32, tag="scm3", bufs=2)
                nc.vector.tensor_scalar_mul(scm[:, 0:nt], sc[:, 0:nt],
                                            NEG_SLOPE)
                nc.vector.tensor_tensor(out=scl[:, 0:nt], in0=scm[:, 0:nt],
                                        in1=sc[:, 0:nt],
                                        op=mybir.AluOpType.max)
                nc.scalar.activation(excol[:, 0:nt], scl[:, 0:nt],
                                     mybir.ActivationFunctionType.Exp)
                B = bp.tile([128, BT * 41], BF16, tag="B3")
                Bv = B[:].rearrange("p (k d) -> p k d", d=41)[:, 0:nt, :]
                nc.vector.tensor_copy(
                    Bv[:, :, 40:41],
                    excol[:].rearrange("p (k d) -> p k d", d=1)[:, 0:nt, :])
                hw = apx(B[:], 0, [[41, nt], [1, 40]])
                hi = apx(T[:], 0, [[128, nt], [1, 40]])
                ex1 = apx(esc[:], 3 * BT, [[1, nt], [0, 40]])
                nc.vector.tensor_tensor(out=hw, in0=hi, in1=ex1,
                                        op=mybir.AluOpType.mult)
                Ssb = sp.tile([128, BT * 128], BF16, tag="S3")
                gen_S(nc, Ssb, dcolsb, iotaP, t0, nt)
                for k in range(nt):
                    tg = t0 + k
                    _, b, tseg = seg_of_tile[tg]
                    tin = tile_in_seg[tg]
                    if tin == 0:
                        ps_cur[b] = pp.tile([128, 41], F32, tag="ps3", name="ps3")
                    ps = ps_cur[b]
                    nc.tensor.matmul(
                        ps[:], Ssb[:, k * 128:(k + 1) * 128],
                        B[:, k * 41:(k + 1) * 41],
                        start=(tin == 0), stop=(tin == tseg - 1))
                    if tin == tseg - 1:
                        nc.vector.tensor_tensor(
                            out=Av[:, b * 41:(b + 1) * 41],
                            in0=Av[:, b * 41:(b + 1) * 41],
                            in1=ps[:], op=mybir.AluOpType.add)
                        del ps_cur[b]
                        if seg_idx_of_tile[tg] == last_seg[b]:
                            node_block3(b)
            assert not ps_cur
            for b in range(NB):
                if b not in last_seg:
                    node_block3(b)

        Ovv = O[:].rearrange("p (b f) -> p b f", f=40)
        mx = no_.tile([128, NB], F32, tag="mx")
        nc.vector.tensor_reduce(mx[:], Ovv, axis=mybir.AxisListType.X,
                                op=mybir.AluOpType.max)
        mxb = apx(mx[:], 0, [[1, NB], [0, 40]])
        nc.vector.tensor_tensor(out=Ovv, in0=Ovv, in1=mxb,
                                op=mybir.AluOpType.subtract)
        E = no_.tile([128, NB * 40], F32, tag="E")
        nc.scalar.activation(E[:], O[:], mybir.ActivationFunctionType.Exp)
        ss = no_.tile([128, NB], F32, tag="ss")
        nc.vector.tensor_reduce(ss[:], E[:].rearrange("p (b f) -> p b f", f=40),
                                axis=mybir.AxisListType.X, op=mybir.AluOpType.add)
        nc.scalar.activation(ss[:], ss[:], mybir.ActivationFunctionType.Ln)
        ssb = apx(ss[:], 0, [[1, NB], [0, 40]])
        nc.vector.tensor_tensor(out=Ovv, in0=Ovv, in1=ssb,
                                op=mybir.AluOpType.subtract)
        nc.sync.dma_start(OUTT[:, :].rearrange("(b p) f -> p b f", p=128), Ovv)


def make_in_maps(plan, weights, features):
    """Per-core input dicts."""
    C = plan["n_cores"]; NL = plan["NL"]
    features = np.asarray(features, np.float32).astype(BF)
    maps = []
    for c in range(C):
        cd = plan["core_data"][c]
        maps.append(dict(
            featT=np.ascontiguousarray(features[c * NL:(c + 1) * NL].T),
            W1ext=weights["W1ext"], W2ext=weights["W2ext"], W3ext=weights["W3ext"],
            b1rep=weights["b1rep"], b2rep=weights["b2rep"], b3rep=weights["b3rep"],
            al1rep=weights["al1rep"], al2rep=weights["al2rep"],
            iota=weights["iota"],
            src16=cd["src16"], dcol=cd["dcol"], S2=cd["S2"],
        ))
    return maps


def assemble_output(plan, results):
    C = plan["n_cores"]; NL = plan["NL"]
    outs = [results[c]["out"][:NL] for c in range(C)]
    return np.concatenate(outs, axis=0)


# ---------------- execution harness (PJRT via bass2jax) ----------------
import jax
from jax.sharding import Mesh, PartitionSpec
from jax.experimental.shard_map import shard_map
from concourse.bass2jax import _bass_exec_p, partition_id_tensor, install_neuronx_cc_hook


def build_runner(nc, n_cores):
    install_neuronx_cc_hook()
    partition_name = nc.partition_id_tensor.name if nc.partition_id_tensor else None
    in_names, out_names, out_avals, zero_outs = [], [], [], []
    in_shapes = []
    for alloc in nc.m.functions[0].allocations:
        if not isinstance(alloc, mybir.MemoryLocationSet):
            continue
        name = alloc.memorylocations[0].name
        if alloc.kind == "ExternalInput":
            if name != partition_name and (nc.dbg_addr is None or name != nc.dbg_addr.name):
                in_names.append(name)
                in_shapes.append((tuple(alloc.tensor_shape), mybir.dt.np(alloc.dtype)))
        elif alloc.kind == "ExternalOutput":
            shape = tuple(alloc.tensor_shape)
            dt = mybir.dt.np(alloc.dtype)
            out_names.append(name)
            out_avals.append(jax.core.ShapedArray(shape, dt))
            zero_outs.append(np.zeros(shape, dt))
    n_params = len(in_names)
    n_outs = len(out_names)
    all_in_names = list(in_names) + list(out_names)
    if nc.dbg_addr is not None:
        all_in_names.append(nc.dbg_addr.name)
    if partition_name is not None:
        all_in_names.append(partition_name)

    def _body(*args):
        operands = list(args)
        if nc.dbg_addr is not None:
            operands.append(jax.numpy.zeros((1, 2), jax.numpy.uint32))
        if partition_name is not None:
            operands.append(partition_id_tensor())
        outs = _bass_exec_p.bind(
            *operands,
            out_avals=tuple(out_avals),
            in_names=tuple(all_in_names),
            out_names=tuple(out_names),
            lowering_input_output_aliases=(),
            sim_require_finite=True,
            sim_require_nnan=True,
            nc=nc,
        )
        return tuple(outs)

    devices = jax.devices()[:n_cores]
    mesh = Mesh(np.asarray(devices), ("core",))
    in_specs = (PartitionSpec("core"),) * (n_params + n_outs)
    out_specs = (PartitionSpec("core"),) * n_outs
    sharded = jax.jit(
        shard_map(_body, mesh=mesh, in_specs=in_specs, out_specs=out_specs,
                  check_rep=False),
        keep_unused=True)
    zeros_concat = [np.zeros((n_cores * z.shape[0], *z.shape[1:]), z.dtype)
                    for z in zero_outs]

    from jax.sharding import NamedSharding
    shard = NamedSharding(mesh, PartitionSpec("core"))
    zeros_dev = jax.device_put(zeros_concat, [shard] * len(zeros_concat)) if zeros_concat else []

    in_avals = [jax.ShapeDtypeStruct((n_cores * s[0], *s[1:]), dt, sharding=shard)
                for s, dt in in_shapes]
    out_zero_avals = [jax.ShapeDtypeStruct(z.shape, z.dtype, sharding=shard)
                      for z in zeros_concat]
    compiled = sharded.lower(*in_avals, *out_zero_avals).compile()

    def fn(concat_inputs):
        return compiled(*concat_inputs, *zeros_dev)

    def put(concat_inputs):
        return jax.device_put(concat_inputs, [shard] * len(concat_inputs))

    return fn, in_names, out_names, put, compiled


_CACHE = {}
_LAST = {}


def _get_compiled(plan_key, plan):
    if plan_key not in _CACHE:
        nc = build_program(plan)
        fn, in_names, out_names, put, compiled = build_runner(nc, plan["n_cores"])
        _CACHE[plan_key] = (nc, fn, in_names, out_names, put, compiled)
    return _CACHE[plan_key]


def run_gat(features, weights_kw, src, dst, n_cores=8, n_timing=0):
    n_nodes = features.shape[0]
    plan = host_preprocess(src, dst, n_nodes, n_cores=n_cores, n_chunks=4)
    weights = host_weights(**weights_kw)
    key = (n_nodes, n_cores, bytes(np.asarray(src[:64]).tobytes()),
           plan["n_tiles"])
    nc, fn, in_names, out_names, put, compiled = _get_compiled(key, plan)
    in_maps = make_in_maps(plan, weights, features)
    concat_in = [np.concatenate([np.asarray(in_maps[c][nm])
                                 for c in range(n_cores)], axis=0)
                 for nm in in_names]
    concat_in = put(concat_in)
    _LAST.update(nc=nc, fn=fn, concat_in=concat_in, plan=plan,
                 compiled=compiled, in_names=in_names, out_names=out_names)
    out = fn(concat_in)
    jax.block_until_ready(out)
    times = []
    if n_timing:
        import time
        for _ in range(n_timing):
            t0 = time.perf_counter()
            out = fn(concat_in)
            jax.block_until_ready(out)
            times.append(time.perf_counter() - t0)
    oi = out_names.index("out")
    arr = np.asarray(out[oi])
    NLP = arr.shape[0] // n_cores
    results = [{"out": arr[c * NLP:(c + 1) * NLP]} for c in range(n_cores)]
    full = assemble_output(plan, results)[:n_nodes]
    return full, times


def kernel(features, W1, al1, ar1, b1, W2, al2, ar2, b2, W3, al3, ar3, b3,
           src, dst):
    wk = dict(W1=W1, al1=al1, ar1=ar1, b1=b1, W2=W2, al2=al2, ar2=ar2, b2=b2,
              W3=W3, al3=al3, ar3=ar3, b3=b3)
    out, _ = run_gat(np.asarray(features, np.float32), wk,
                     np.asarray(src), np.asarray(dst), n_cores=8)
    return out.astype(np.float32)


# revision 29
# speedup vs baseline: 10.9895x; 10.9895x over previous
"""GAT Trainium kernel v3: 256B gather rows + PSUM-decontended DVE ops.

v3 changes vs v2 (driven by NTFF profile: GpSimd 86% busy, 3.65ms in
DMAGatherAnt at ~7.8ns/idx while SDMA queues were only ~20% busy):
- The Q7 descriptor generator shares an SBUF port pair with DVE; every
  2-input DVE op (is_equal, multiply, leaky) locked GpSimd out for its whole
  duration. All hot DVE ops are now 1-SBUF-port: one operand lives in PSUM
  (iota for gen_S, exp/scores in PSUM), leaky-relu + exp moved to the ACT
  engine (Lrelu/Exp), memset of the accumulator moved to ACT.
- G rows shrunk 512B -> 256B (128 bf16 = h only). el is no longer carried in
  the row; the consumer computes el = sum(h*al) per head with a DVE mult +
  tensor_reduce per batch. Halves gather bytes and AllGather wire volume.
- Layer-3 rows stay 256B: [h x40 | el x1 | pad] bf16.
"""
from contextlib import ExitStack
import numpy as np
import ml_dtypes
import concourse.bass as bass
import concourse.tile as tile
from concourse import bacc, mybir
from concourse.masks import make_identity

F32 = mybir.dt.float32
BF16 = mybir.dt.bfloat16
I16 = mybir.dt.int16
BF = ml_dtypes.bfloat16

IN, HID, HEADS, OUT = 256, 32, 4, 40
HH = HID * HEADS  # 128
NEG_SLOPE = 0.2
BT = 16  # tiles per gather batch (16*128 = 2048 idx/call)


def wrap16(a):
    n = a.shape[0]
    assert n % 16 == 0
    blk = a.reshape(-1, 16).T  # [16, n/16]
    return np.tile(blk, (8, 1)).astype(np.int16)


def _balance_positions(indeg, NB, NCH, QR, NL):
    """Assign NL local nodes to positions, keeping each node inside its
    original quarter (so src-chunk assignments stay fixed) while packing
    nodes into 128-row blocks to minimize the max per-(block, chunk)
    in-edge count (the SPMD capacity every core pads to)."""
    pos = np.empty(NL, np.int64)
    load = np.zeros((NB, NCH), np.float64)
    tot = np.zeros(NB, np.float64)
    quarter = np.arange(NL) // QR
    for v in range(NCH):
        p0, p1 = v * QR, (v + 1) * QR
        nodes = np.nonzero(quarter == v)[0]
        b_lo, b_hi = p0 // 128, (p1 - 1) // 128
        bins = np.arange(b_lo, b_hi + 1)
        slots = (np.minimum((bins + 1) * 128, p1)
                 - np.maximum(bins * 128, p0))
        nxt = np.maximum(bins * 128, p0).copy()
        order = nodes[np.argsort(-indeg[nodes].sum(1), kind="stable")]
        for n in order:
            w = indeg[n]
            cand = (load[bins] + w).max(1) + 1e-3 * tot[bins]
            cand[slots == 0] = 1e18
            j = int(cand.argmin())
            b = int(bins[j])
            pos[n] = nxt[j]
            nxt[j] += 1
            slots[j] -= 1
            load[b] += w
            tot[b] += w.sum()
    return pos


def host_preprocess(src, dst, n_nodes, n_cores=8, n_chunks=4):
    NL = n_nodes // n_cores
    assert NL * n_cores == n_nodes
    NB = (NL + 127) // 128
    CS = (n_nodes + n_chunks - 1) // n_chunks
    assert CS <= 32767
    QR = NL // n_chunks
    assert QR * n_chunks == NL
    cap_last = NL - 128 * (NB - 1)

    src = np.asarray(src, np.int64); dst = np.asarray(dst, np.int64)
    core_of = dst // NL

    # --- per-core node->position balancing ---
    # position determines BOTH the node's dst block (pos//128) and its src
    # chunk (pos//QR). Nodes stay inside their original quarter (chunk map
    # fixed, so no cross-core fixed-point issues) while block packing
    # minimizes the max-over-cores (block, chunk) cell count.
    _ = cap_last
    chunk_map = (np.arange(n_nodes) % NL) // QR
    pos_of = np.empty(n_nodes, np.int64)
    for c in range(n_cores):
        m = core_of == c
        dl = dst[m] - c * NL
        ch = chunk_map[src[m]]
        indeg = np.zeros((NL, n_chunks), np.int64)
        np.add.at(indeg, (dl, ch), 1)
        pos_of[c * NL:(c + 1) * NL] = _balance_positions(
            indeg, NB, n_chunks, QR, NL)

    # --- per-core edge lists in (chunk, block) order ---
    per_core = []
    counts = np.zeros((n_cores, NB, n_chunks), np.int64)
    for c in range(n_cores):
        m = core_of == c
        s, d = src[m], dst[m]
        dpos = pos_of[d]            # local position on this core
        b = dpos // 128
        dmod = dpos % 128
        cs_ = s // NL
        spos = pos_of[s]            # local position on src's owner core
        ch = spos // QR
        sloc = cs_ * QR + (spos % QR)   # row within the chunk's gather table
        order = np.lexsort((dmod, b, ch))
        per_core.append((sloc[order], b[order], dmod[order], ch[order]))
        np.add.at(counts, (np.full(b.shape, c), b, ch), 1)
    # capacities rounded to 32 so piece boundaries land on legal PE
    # tile_position bases (0/32/64/96)
    cap = ((counts.max(axis=0) + 31) // 32) * 32  # [NB, n_chunks]

    # --- straddle-packed segments: capacity cap[b,q], packed back to back
    # within each chunk; tiles of 128 may straddle segment boundaries ---
    segs = []          # (chunk, block, g0, capn) in global slot coords
    chunk_tiles = []
    g = 0
    tile_base = []
    for q in range(n_chunks):
        cbase = g
        for b in range(NB):
            n = int(cap[b, q])
            if n > 0:
                segs.append((q, b, g, n))
            g += n
        g = ((g + 127) // 128) * 128
        chunk_tiles.append((g - cbase) // 128)
    total_slots = g
    n_tiles = total_slots // 128

    last_seg_of_block = {}
    for si, (q, b, g0, n) in enumerate(segs):
        last_seg_of_block[b] = si

    # pieces per tile: (e0, e1, seg_idx, first, last), split into legal PE
    # tiles: base 0 any size; base 32/96 size<=32; base 64 size<=64
    def pe_split(e0, e1):
        out = []
        while e0 < e1:
            if e0 == 0:
                e = e1
            elif e0 == 32:
                e = min(e1, 64)
            elif e0 == 64:
                e = min(e1, 128)
            else:
                assert e0 == 96
                e = e1
            out.append((e0, e))
            e0 = e
        return out

    tile_pieces = [[] for _ in range(n_tiles)]
    for si, (q, b, g0, n) in enumerate(segs):
        g1 = g0 + n
        t0, t1 = g0 // 128, (g1 - 1) // 128
        for t in range(t0, t1 + 1):
            e0 = max(g0, t * 128) - t * 128
            e1 = min(g1, (t + 1) * 128) - t * 128
            subs = pe_split(e0, e1)
            for j, (s0, s1) in enumerate(subs):
                first = t * 128 + s0 == g0
                last = t * 128 + s1 == g1
                tile_pieces[t].append((s0, s1, si, first, last))

    batches = []
    t = 0
    for q in range(n_chunks):
        nt_q = chunk_tiles[q]
        done = 0
        while done < nt_q:
            nt = min(BT, nt_q - done)
            batches.append({"chunk": q, "t0": t, "nt": nt})
            t += nt
            done += nt
    assert t == n_tiles

    core_data = []
    seg_arr = segs
    for c in range(n_cores):
        sloc, b_arr, dmod_arr, ch_arr = per_core[c]
        src16 = np.zeros(total_slots, np.int16)
        dcol = np.full((128, n_tiles), -1.0, np.float32)  # pad never matches
        S2 = np.zeros((128, total_slots), BF)  # [d, t*128+e] one-hot
        # edges sorted by (ch, b); segments sorted the same way -> walk
        ptr = 0
        for q, bb, g0, capn in seg_arr:
            # count this core's edges for (q, bb): they are contiguous at ptr
            n = 0
            while ptr + n < len(b_arr) and ch_arr[ptr + n] == q and b_arr[ptr + n] == bb:
                n += 1
            assert n <= capn
            if n:
                sl = sloc[ptr:ptr + n]
                dm = dmod_arr[ptr:ptr + n]
                slot = g0 + np.arange(n)
                src16[slot] = sl
                dcol[slot % 128, slot // 128] = dm
                S2[dm, slot] = 1.0
            ptr += n
        assert ptr == len(b_arr)
        core_data.append(dict(
            src16=wrap16(src16),
            dcol=dcol,
            S2=S2,
            pos=np.argsort(pos_of[c * NL:(c + 1) * NL], kind="stable"),
            pos_of=pos_of[c * NL:(c + 1) * NL].copy(),
        ))

    return dict(
        n_cores=n_cores, n_nodes=n_nodes, NL=NL, NB=NB, CS=CS, QR=QR,
        n_chunks=n_chunks, segs=segs, batches=batches, n_tiles=n_tiles,
        tile_pieces=tile_pieces, last_seg_of_block=last_seg_of_block,
        total_slots=total_slots, core_data=core_data,
    )


def host_weights(W1, al1, ar1, b1, W2, al2, ar2, b2, W3, al3, ar3, b3):
    def bd(al):
        al = np.asarray(al, np.float32)
        H, F = al.shape
        out = np.zeros((H * F, H), np.float32)
        for h in range(H):
            out[h * F:(h + 1) * F, h] = al[h]
        return out
    W1 = np.asarray(W1, np.float32); W2 = np.asarray(W2, np.float32); W3 = np.asarray(W3, np.float32)
    # producer rows carry only h (+er table); el computed on the consumer
    W1ext = np.concatenate([W1, W1 @ bd(ar1)], axis=1)            # [IN, 132]
    W2ext = np.concatenate([W2, W2 @ bd(ar2)], axis=1)            # [HH, 132]
    W3ext = np.concatenate([W3, W3 @ bd(al3), W3 @ bd(ar3)], axis=1)  # [HH, 42]
    b1rep = np.tile(np.asarray(b1, np.float32).reshape(1, HH), (128, 1))
    b2rep = np.tile(np.asarray(b2, np.float32).reshape(1, HH), (128, 1))
    b3rep = np.tile(np.asarray(b3, np.float32).reshape(1, OUT), (128, 1))
    iota = np.tile(np.arange(128, dtype=np.float32)[None, :], (128, 1))
    al1rep = np.tile(np.asarray(al1, np.float32).reshape(1, HH), (128, 1))
    al2rep = np.tile(np.asarray(al2, np.float32).reshape(1, HH), (128, 1))
    return dict(W1ext=W1ext.astype(BF), W2ext=W2ext.astype(BF),
                W3ext=W3ext.astype(BF),
                b1rep=b1rep, b2rep=b2rep, b3rep=b3rep,
                iota=iota,
                al1rep=al1rep.astype(BF), al2rep=al2rep.astype(BF))


def apx(base_ap, col_off, dims):
    """AP at column offset of a [128, W] tile with custom free dims."""
    b = base_ap[:, col_off:col_off + 1]
    return bass.AP(b.tensor, b.offset, [b.ap[0]] + [list(d) for d in dims])


def build_program(plan, stage=99):
    C = plan["n_cores"]; NL = plan["NL"]; NB = plan["NB"]
    NT = plan["n_tiles"]; TS = plan["total_slots"]
    NLP = NB * 128

    nc = bacc.Bacc("TRN2", target_bir_lowering=False, debug=False, num_devices=C)

    featT = nc.dram_tensor("featT", [IN, NL], BF16, kind="ExternalInput").ap()
    W1e = nc.dram_tensor("W1ext", [IN, 132], BF16, kind="ExternalInput").ap()
    W2e = nc.dram_tensor("W2ext", [HH, 132], BF16, kind="ExternalInput").ap()
    W3e = nc.dram_tensor("W3ext", [HH, 42], BF16, kind="ExternalInput").ap()
    B1 = nc.dram_tensor("b1rep", [128, HH], F32, kind="ExternalInput").ap()
    B2 = nc.dram_tensor("b2rep", [128, HH], F32, kind="ExternalInput").ap()
    B3 = nc.dram_tensor("b3rep", [128, OUT], F32, kind="ExternalInput").ap()
    AL1 = nc.dram_tensor("al1rep", [128, HH], BF16, kind="ExternalInput").ap()
    AL2 = nc.dram_tensor("al2rep", [128, HH], BF16, kind="ExternalInput").ap()
    SRC = nc.dram_tensor("src16", [128, TS // 16], I16, kind="ExternalInput").ap()
    DCOL = nc.dram_tensor("dcol", [128, NT], F32, kind="ExternalInput").ap()
    IOTA = nc.dram_tensor("iota", [128, 128], F32, kind="ExternalInput").ap()
    S2D = nc.dram_tensor("S2", [128, TS], BF16, kind="ExternalInput").ap()
    OUTT = nc.dram_tensor("out", [NLP, OUT], F32, kind="ExternalOutput").ap()

    G1s = nc.dram_tensor("G1slab", [NL, 128], BF16).ap()
    G2s = nc.dram_tensor("G2slab", [NL, 128], BF16).ap()
    G3s = nc.dram_tensor("G3slab", [NL, 128], BF16).ap()
    G1 = nc.dram_tensor("G1", [C * NL, 128], BF16, addr_space="Shared").ap()
    G2 = nc.dram_tensor("G2", [C * NL, 128], BF16, addr_space="Shared").ap()
    G3 = nc.dram_tensor("G3", [C * NL, 128], BF16, addr_space="Shared").ap()

    rg = [list(range(C))]

    QR = plan["QR"]; NQ = plan["n_chunks"]; CSz = plan["CS"]

    def allgather(slab, full):
        # quarter-interleaved: AG of slab rows [q*QR,(q+1)*QR) from all cores
        # completes gather-chunk q of `full` -> edge phase chunk q can start
        # while later quarters are still gathering
        for q in range(NQ):
            nc.gpsimd.collective_compute(
                "AllGather", mybir.AluOpType.bypass,
                replica_groups=rg,
                ins=[slab[q * QR:(q + 1) * QR, :]],
                outs=[full[q * CSz:(q + 1) * CSz, :]])

    with tile.TileContext(nc) as tc, ExitStack() as ctx:
        const = ctx.enter_context(tc.tile_pool(name="const", bufs=1))
        accp = ctx.enter_context(tc.tile_pool(name="acc", bufs=1))
        constp = ctx.enter_context(tc.tile_pool(name="constp", bufs=1, space="PSUM"))

        w1sb = const.tile([128, 2 * 132], BF16)
        nc.sync.dma_start(w1sb[:, 0:132], W1e[0:128, :])
        nc.sync.dma_start(w1sb[:, 132:264], W1e[128:256, :])
        w2sb = const.tile([128, 132], BF16)
        nc.sync.dma_start(w2sb[:], W2e[:, :])
        w3sb = const.tile([128, 42], BF16)
        nc.sync.dma_start(w3sb[:], W3e[:, :])
        b1sb = const.tile([128, HH], F32)
        nc.sync.dma_start(b1sb[:], B1[:, :])
        b2sb = const.tile([128, HH], F32)
        nc.sync.dma_start(b2sb[:], B2[:, :])
        b3sb = const.tile([128, OUT], F32)
        nc.sync.dma_start(b3sb[:], B3[:, :])
        al1sb = const.tile([128, HH], BF16)
        nc.sync.dma_start(al1sb[:], AL1[:, :])
        al2sb = const.tile([128, HH], BF16)
        nc.sync.dma_start(al2sb[:], AL2[:, :])
        ident = const.tile([128, 128], F32)
        make_identity(nc, ident[:])
        iotasb = const.tile([128, 128], F32)
        nc.sync.dma_start(iotasb[:], IOTA[:, :])
        dcolsb = const.tile([128, NT], F32)
        nc.sync.dma_start(dcolsb[:], DCOL[:, :])
        srcsb = const.tile([128, TS // 16], I16)
        nc.sync.dma_start(srcsb[:], SRC[:, :])
        # iota parked in PSUM so gen_S is a 1-SBUF-port DVE op
        iotaP = constp.tile([128, 128], F32)
        nc.vector.tensor_copy(iotaP[:], iotasb[:])
        # SBUF er tables, one per layer: [128 (node-in-block), NB*heads]
        er1sb = const.tile([128, NB * 4], BF16)
        er2sb = const.tile([128, NB * 4], BF16)
        er3sb = const.tile([128, NB], BF16)

        def write_node_rows(lp, ps, nr, b, r0, r1, Gn_s, ersb, n_h, n_el):
            """ps [nr, n_h+n_el+n_er] f32 PSUM -> G slab row bf16 + er table."""
            gsb = lp.tile([128, 128], BF16, tag="gsb")
            if n_el:  # layer-3 rows carry el packed after h
                nc.vector.tensor_copy(gsb[:nr, 0:n_h + n_el], ps[:nr, 0:n_h + n_el])
            else:
                nc.vector.tensor_copy(gsb[:nr, 0:n_h], ps[:nr, 0:n_h])
            nc.sync.dma_start(Gn_s[r0:r1, 0:n_h + n_el], gsb[:nr, 0:n_h + n_el])
            ner = ersb.shape[1] // NB
            nc.vector.tensor_copy(ersb[:nr, b * ner:(b + 1) * ner],
                                  ps[:nr, n_h + n_el:n_h + n_el + ner])

        # Layer 1 node phase
        with tc.tile_pool(name="l1n", bufs=3) as lp, \
             tc.tile_pool(name="l1np", bufs=2, space="PSUM") as pp:
            for b in range(NB):
                r0 = b * 128
                r1 = min(r0 + 128, NL)
                nr = r1 - r0
                xt = lp.tile([128, 256], BF16, tag="xt")
                nc.sync.dma_start(xt[:, 0:nr], featT[0:128, r0:r1])
                nc.sync.dma_start(xt[:, 128:128 + nr], featT[128:256, r0:r1])
                ps = pp.tile([128, 132], F32, tag="ps")
                nc.tensor.matmul(ps[:nr, :], xt[:, 0:nr], w1sb[:, 0:132],
                                 start=True, stop=False)
                nc.tensor.matmul(ps[:nr, :], xt[:, 128:128 + nr], w1sb[:, 132:264],
                                 start=False, stop=True)
                write_node_rows(lp, ps, nr, b, r0, r1, G1s, er1sb, 128, 0)

        if stage >= 2:
            allgather(G1s, G1)
        if stage >= 3:
            edge_layer(tc, plan, 1, G1, er1sb, srcsb, dcolsb, iotaP, S2D, accp,
                       w_next=w2sb, b_rep=b1sb, Gn_s=G2s, ersb_n=er2sb,
                       ident=ident, write_node=write_node_rows, alsb=al1sb,
                       node_phase=(stage >= 4))
        if stage >= 5:
            allgather(G2s, G2)
            edge_layer(tc, plan, 2, G2, er2sb, srcsb, dcolsb, iotaP, S2D, accp,
                       w_next=w3sb, b_rep=b2sb, Gn_s=G3s, ersb_n=er3sb,
                       ident=ident, write_node=write_node_rows, alsb=al2sb)
            allgather(G3s, G3)
        if stage >= 6:
            edge_layer3(tc, plan, G3, er3sb, srcsb, dcolsb, iotaP, S2D, accp,
                        b3sb, ident, OUTT)

    nc.compile()
    return nc





def gen_S(nc, Ssb, dcolsb, iotaP, t0, nt):
    """S[e, k, d] = (dcol[e, t0+k] == iota[d]), bf16.

    dcol read from SBUF, iota from PSUM -> single SBUF read port, so the op
    does not lock GpSimd out of the shared port pair during desc-gen.
    """
    Sv = apx(Ssb[:], 0, [[128, nt], [1, 128]])
    dc = apx(dcolsb[:], t0, [[1, nt], [0, 128]])
    io = apx(iotaP[:], 0, [[0, nt], [1, 128]])
    nc.vector.tensor_tensor(out=Sv, in0=dc, in1=io,
                            op=mybir.AluOpType.is_equal)


def edge_layer(tc, plan, lnum, G, ersb, srcsb, dcolsb, iotaP, S2D, accp,
               w_next, b_rep, Gn_s, ersb_n, ident, write_node, alsb,
               node_phase=True):
    nc = tc.nc
    NL = plan["NL"]; NB = plan["NB"]; CS = plan["CS"]
    segs = plan["segs"]; pieces = plan["tile_pieces"]
    last_seg = plan["last_seg_of_block"]

    A = accp.tile([128, NB * 132], F32, tag="A")
    nc.scalar.memzero(A[:])

    NOUT = w_next.shape[1]

    with tc.tile_pool(name=f"e{lnum}", bufs=4) as ep, \
         tc.tile_pool(name=f"e{lnum}b", bufs=3) as bp, \
         tc.tile_pool(name=f"e{lnum}s", bufs=4) as sp, \
         tc.tile_pool(name=f"n{lnum}", bufs=3) as np_, \
         tc.tile_pool(name=f"e{lnum}p", bufs=2, space="PSUM") as pp, \
         tc.tile_pool(name=f"e{lnum}e", bufs=2, space="PSUM") as pe, \
         tc.tile_pool(name=f"n{lnum}p", bufs=1, space="PSUM") as npp:

        def node_block(b):
            if not node_phase:
                return
            r0 = b * 128
            r1 = min(r0 + 128, NL)
            nr = r1 - r0
            Ab = A[:, b * 132:(b + 1) * 132]
            # nodeA bank: [0:128) hv, [128:132) rs
            nodeA = npp.tile([128, 132], F32, tag="nodeA")
            rs = nodeA[:, 128:132]
            hv = nodeA[:, 0:128]
            nc.vector.tensor_scalar_max(rs, Ab[:, 128:132], 1e-30)
            nc.vector.reciprocal(rs, rs)
            hvv = hv.rearrange("p (g f) -> p g f", g=4)
            rsb = apx(nodeA[:], 128, [[1, 4], [0, 32]])
            av = Ab[:, 0:128].rearrange("p (g f) -> p g f", g=4)
            nc.vector.tensor_tensor(out=hvv, in0=av, in1=rsb,
                                    op=mybir.AluOpType.mult)
            hp = np_.tile([128, 128], F32, tag="hp")
            nc.vector.tensor_tensor(out=hp[:], in0=hv, in1=b_rep[:],
                                    op=mybir.AluOpType.add)
            nc.scalar.activation(hp[:], hp[:], mybir.ActivationFunctionType.Relu)
            pst = npp.tile([128, 128], F32, tag="pst")
            nc.tensor.transpose(out=pst[:], in_=hp[:], identity=ident[:])
            hpt = np_.tile([128, 128], BF16, tag="hpt")
            nc.vector.tensor_copy(hpt[:], pst[:])
            ps2 = npp.tile([128, NOUT], F32, tag="ps2")
            nc.tensor.matmul(ps2[:nr, :], hpt[:, 0:nr], w_next[:],
                             start=True, stop=True)
            n_el_next = 1 if NOUT == 42 else 0
            n_h_next = 40 if NOUT == 42 else 128
            write_node(np_, ps2, nr, b, r0, r1, Gn_s, ersb_n, n_h_next, n_el_next)

        ps_cur = {}
        for bt in plan["batches"]:
            nt = bt["nt"]; t0 = bt["t0"]; chh = bt["chunk"]
            nidx = nt * 128
            T = ep.tile([128, BT * 128], BF16, tag="T")
            Tv = T[:].rearrange("p (k d) -> p k d", d=128)[:, 0:nt, :]
            nc.gpsimd.dma_gather(
                Tv, G[chh * CS:(chh + 1) * CS, :],
                srcsb[:, t0 * 8:(t0 + nt) * 8],
                nidx, nidx, 128, single_packet=False)
            # S2 one-hot [d, e] stream for er matmuls
            S2sb = sp.tile([128, BT * 128], BF16, tag="S2")
            nc.sync.dma_start(S2sb[:, 0:nt * 128],
                              S2D[:, t0 * 128:(t0 + nt) * 128])
            # escore bank: [0:4BT) erps, [4BT:8BT) sc, [8BT:12BT) leaky, [12BT:16BT) exp
            esc = pe.tile([128, BT * 16], F32, tag="esc")
            erps = esc[:, 0:BT * 4]
            sc = esc[:, BT * 4:BT * 8]
            scl = esc[:, BT * 8:BT * 12]
            excol = esc[:, BT * 12:BT * 16]
            # er[e, (k,h)] = S2_piece.T @ erb_block (lands in score layout)
            for k in range(nt):
                for e0, e1, si, _, _ in pieces[t0 + k]:
                    _, b, _, _ = segs[si]
                    nc.tensor.matmul(
                        erps[e0:e1, k * 4:(k + 1) * 4],
                        S2sb[:, k * 128 + e0:k * 128 + e1],
                        ersb[:, b * 4:(b + 1) * 4],
                        start=True, stop=True, tile_position=(0, e0))
            # el[e, (k,h)] = sum_f T[e,(k,h,f)] * al[(h,f)] : mult + reduce
            eltmp = ep.tile([128, BT * 128], BF16, tag="eltmp")
            etv = eltmp[:].rearrange("p (k d) -> p k d", d=128)[:, 0:nt, :]
            alv = apx(alsb[:], 0, [[0, nt], [1, 128]])
            Tvb = T[:].rearrange("p (k d) -> p k d", d=128)[:, 0:nt, :]
            nc.vector.tensor_tensor(out=etv, in0=Tvb, in1=alv,
                                    op=mybir.AluOpType.mult)
            elsb = np_.tile([128, BT * 4], F32, tag="elsb")
            elv3 = eltmp[:].rearrange("p (k g f) -> p (k g) f", g=4, f=32)[:, 0:nt * 4, :]
            nc.vector.tensor_reduce(elsb[:, 0:nt * 4], elv3,
                                    axis=mybir.AxisListType.X,
                                    op=mybir.AluOpType.add)
            # score = leaky(el + er); el SBUF + er PSUM -> sc PSUM (1-port)
            nc.vector.tensor_tensor(out=sc[:, 0:nt * 4], in0=elsb[:, 0:nt * 4],
                                    in1=erps[:, 0:nt * 4],
                                    op=mybir.AluOpType.add)
            # leaky = max(0.2*sc, sc): two 1-PSUM-read DVE ops (DVE may read
            # only one PSUM input per instruction)
            scm = np_.tile([128, BT * 4], F32, tag="scm")
            nc.vector.tensor_scalar_mul(scm[:, 0:nt * 4], sc[:, 0:nt * 4],
                                        NEG_SLOPE)
            nc.vector.tensor_tensor(out=scl[:, 0:nt * 4], in0=scm[:, 0:nt * 4],
                                    in1=sc[:, 0:nt * 4],
                                    op=mybir.AluOpType.max)
            nc.scalar.activation(excol[:, 0:nt * 4], scl[:, 0:nt * 4],
                                 mybir.ActivationFunctionType.Exp)
            B = bp.tile([128, BT * 132], BF16, tag="B")
            Bv = B[:].rearrange("p (k d) -> p k d", d=132)[:, 0:nt, :]
            # alpha into B's payload cols (for the denominator row of ps)
            nc.vector.tensor_copy(Bv[:, :, 128:132],
                                  excol[:].rearrange("p (k d) -> p k d", d=4)[:, 0:nt, :])
            # weighted h: T (SBUF) * exp (PSUM broadcast) -> B (1 SBUF read)
            hw = apx(B[:], 0, [[132, nt], [32, 4], [1, 32]])
            hi = apx(T[:], 0, [[128, nt], [32, 4], [1, 32]])
            ex4 = apx(esc[:], BT * 12, [[4, nt], [1, 4], [0, 32]])
            nc.vector.tensor_tensor(out=hw, in0=hi, in1=ex4,
                                    op=mybir.AluOpType.mult)
            Ssb = sp.tile([128, BT * 128], BF16, tag="S")
            gen_S(nc, Ssb, dcolsb, iotaP, t0, nt)
            for k in range(nt):
                for e0, e1, si, first, last in pieces[t0 + k]:
                    _, b, _, _ = segs[si]
                    if first:
                        ps_cur[si] = pp.tile([128, 132], F32, tag="ps", name="ps")
                    ps = ps_cur[si]
                    nc.tensor.matmul(
                        ps[:], Ssb[e0:e1, k * 128:(k + 1) * 128],
                        B[e0:e1, k * 132:(k + 1) * 132],
                        start=first, stop=last, tile_position=(e0, 0))
                    if last:
                        nc.vector.tensor_tensor(
                            out=A[:, b * 132:(b + 1) * 132],
                            in0=A[:, b * 132:(b + 1) * 132],
                            in1=ps[:], op=mybir.AluOpType.add)
                        del ps_cur[si]
                        if si == last_seg[b]:
                            node_block(b)
        assert not ps_cur
        for b in range(NB):
            if b not in last_seg:
                node_block(b)


def edge_layer3(tc, plan, G, ersb, srcsb, dcolsb, iotaP, S2D, accp,
                b3sb, ident, OUTT):
    nc = tc.nc
    NL = plan["NL"]; NB = plan["NB"]; CS = plan["CS"]
    segs = plan["segs"]; pieces = plan["tile_pieces"]
    last_seg = plan["last_seg_of_block"]

    A = accp.tile([128, NB * 132], F32, tag="A")
    Av = A[:, 0:NB * 41]
    nc.scalar.memzero(A[:])

    with tc.tile_pool(name="n3", bufs=1) as no_, \
         tc.tile_pool(name="n3p", bufs=2, space="PSUM") as nop:
        O = no_.tile([128, NB * 40], F32, tag="O")

        def node_block3(b):
            Ab = Av[:, b * 41:(b + 1) * 41]
            rs = nop.tile([128, 4], F32, tag="rs3")
            nc.vector.tensor_scalar_max(rs[:, 0:1], Ab[:, 40:41], 1e-30)
            nc.vector.reciprocal(rs[:, 0:1], rs[:, 0:1])
            rsb = apx(rs[:], 0, [[0, 40]])
            Ob = O[:, b * 40:(b + 1) * 40]
            nc.vector.tensor_tensor(out=Ob, in0=Ab[:, 0:40], in1=rsb,
                                    op=mybir.AluOpType.mult)
            nc.vector.tensor_tensor(out=Ob, in0=Ob, in1=b3sb[:],
                                    op=mybir.AluOpType.add)

        with tc.tile_pool(name="e3", bufs=4) as ep, \
             tc.tile_pool(name="e3b", bufs=3) as bp, \
             tc.tile_pool(name="e3s", bufs=4) as sp, \
             tc.tile_pool(name="e3p", bufs=3, space="PSUM") as pp, \
             tc.tile_pool(name="e3e", bufs=2, space="PSUM") as pe:
            ps_cur = {}
            for bt in plan["batches"]:
                nt = bt["nt"]; t0 = bt["t0"]; chh = bt["chunk"]
                nidx = nt * 128
                T = ep.tile([128, BT * 128], BF16, tag="T3")
                Tv = T[:].rearrange("p (k d) -> p k d", d=128)[:, 0:nt, :]
                nc.gpsimd.dma_gather(
                    Tv, G[chh * CS:(chh + 1) * CS, :],
                    srcsb[:, t0 * 8:(t0 + nt) * 8],
                    nidx, nidx, 128, single_packet=False)
                S2sb = sp.tile([128, BT * 128], BF16, tag="S23")
                nc.sync.dma_start(S2sb[:, 0:nt * 128],
                                  S2D[:, t0 * 128:(t0 + nt) * 128])
                # escore bank: [0:BT) erps, [BT:2BT) sc, [2BT:3BT) leaky, [3BT:4BT) exp
                esc = pe.tile([128, BT * 4], F32, tag="esc3")
                erps = esc[:, 0:BT]
                sc = esc[:, BT:2 * BT]
                scl = esc[:, 2 * BT:3 * BT]
                excol = esc[:, 3 * BT:4 * BT]
                for k in range(nt):
                    for e0, e1, si, _, _ in pieces[t0 + k]:
                        _, b, _, _ = segs[si]
                        nc.tensor.matmul(
                            erps[e0:e1, k:k + 1],
                            S2sb[:, k * 128 + e0:k * 128 + e1],
                            ersb[:, b:b + 1],
                            start=True, stop=True, tile_position=(0, e0))
                # el is packed bf16 in the row at col 40
                scv = sc[:].rearrange("p (k d) -> p k d", d=1)[:, 0:nt, :]
                erv = erps[:].rearrange("p (k d) -> p k d", d=1)[:, 0:nt, :]
                nc.vector.tensor_tensor(out=scv, in0=Tv[:, :, 40:41], in1=erv,
                                        op=mybir.AluOpType.add)
                scm = no_.tile([128, BT], F32, tag="scm3", bufs=2)
                nc.vector.tensor_scalar_mul(scm[:, 0:nt], sc[:, 0:nt],
                                            NEG_SLOPE)
                nc.vector.tensor_tensor(out=scl[:, 0:nt], in0=scm[:, 0:nt],
                                        in1=sc[:, 0:nt],
                                        op=mybir.AluOpType.max)
                nc.scalar.activation(excol[:, 0:nt], scl[:, 0:nt],
                                     mybir.ActivationFunctionType.Exp)
                B = bp.tile([128, BT * 41], BF16, tag="B3")
                Bv = B[:].rearrange("p (k d) -> p k d", d=41)[:, 0:nt, :]
                nc.vector.tensor_copy(
                    Bv[:, :, 40:41],
                    excol[:].rearrange("p (k d) -> p k d", d=1)[:, 0:nt, :])
                hw = apx(B[:], 0, [[41, nt], [1, 40]])
                hi = apx(T[:], 0, [[128, nt], [1, 40]])
                ex1 = apx(esc[:], 3 * BT, [[1, nt], [0, 40]])
                nc.vector.tensor_tensor(out=hw, in0=hi, in1=ex1,
                                        op=mybir.AluOpType.mult)
                Ssb = sp.tile([128, BT * 128], BF16, tag="S3")
                gen_S(nc, Ssb, dcolsb, iotaP, t0, nt)
                for k in range(nt):
                    for e0, e1, si, first, last in pieces[t0 + k]:
                        _, b, _, _ = segs[si]
                        if first:
                            ps_cur[si] = pp.tile([128, 41], F32, tag="ps3",
                                                 name="ps3")
                        ps = ps_cur[si]
                        nc.tensor.matmul(
                            ps[:], Ssb[e0:e1, k * 128:(k + 1) * 128],
                            B[e0:e1, k * 41:(k + 1) * 41],
                            start=first, stop=last, tile_position=(e0, 0))
                        if last:
                            nc.vector.tensor_tensor(
                                out=Av[:, b * 41:(b + 1) * 41],
                                in0=Av[:, b * 41:(b + 1) * 41],
                                in1=ps[:], op=mybir.AluOpType.add)
                            del ps_cur[si]
                            if si == last_seg[b]:
                                node_block3(b)
            assert not ps_cur
            for b in range(NB):
                if b not in last_seg:
                    node_block3(b)

        Ovv = O[:].rearrange("p (b f) -> p b f", f=40)
        mx = no_.tile([128, NB], F32, tag="mx")
        nc.vector.tensor_reduce(mx[:], Ovv, axis=mybir.AxisListType.X,
                                op=mybir.AluOpType.max)
        mxb = apx(mx[:], 0, [[1, NB], [0, 40]])
        nc.vector.tensor_tensor(out=Ovv, in0=Ovv, in1=mxb,
                                op=mybir.AluOpType.subtract)
        E = no_.tile([128, NB * 40], F32, tag="E")
        nc.scalar.activation(E[:], O[:], mybir.ActivationFunctionType.Exp)
        ss = no_.tile([128, NB], F32, tag="ss")
        nc.vector.tensor_reduce(ss[:], E[:].rearrange("p (b f) -> p b f", f=40),
                                axis=mybir.AxisListType.X, op=mybir.AluOpType.add)
        nc.scalar.activation(ss[:], ss[:], mybir.ActivationFunctionType.Ln)
        ssb = apx(ss[:], 0, [[1, NB], [0, 40]])
        nc.vector.tensor_tensor(out=Ovv, in0=Ovv, in1=ssb,
                                op=mybir.AluOpType.subtract)
        nc.sync.dma_start(OUTT[:, :].rearrange("(b p) f -> p b f", p=128), Ovv)


def make_in_maps(plan, weights, features):
    """Per-core input dicts."""
    C = plan["n_cores"]; NL = plan["NL"]
    features = np.asarray(features, np.float32).astype(BF)
    maps = []
    for c in range(C):
        cd = plan["core_data"][c]
        maps.append(dict(
            featT=np.ascontiguousarray(features[c * NL:(c + 1) * NL][cd["pos"]].T),
            W1ext=weights["W1ext"], W2ext=weights["W2ext"], W3ext=weights["W3ext"],
            b1rep=weights["b1rep"], b2rep=weights["b2rep"], b3rep=weights["b3rep"],
            al1rep=weights["al1rep"], al2rep=weights["al2rep"],
            iota=weights["iota"],
            src16=cd["src16"], dcol=cd["dcol"], S2=cd["S2"],
        ))
    return maps


def assemble_output(plan, results):
    C = plan["n_cores"]; NL = plan["NL"]
    outs = []
    for c in range(C):
        pos_of = plan["core_data"][c]["pos_of"]  # local node -> position
        outs.append(results[c]["out"][pos_of])
    return np.concatenate(outs, axis=0)


# ---------------- execution harness (PJRT via bass2jax) ----------------
import jax
from jax.sharding import Mesh, PartitionSpec
from jax.experimental.shard_map import shard_map
from concourse.bass2jax import _bass_exec_p, partition_id_tensor, install_neuronx_cc_hook


def build_runner(nc, n_cores):
    install_neuronx_cc_hook()
    partition_name = nc.partition_id_tensor.name if nc.partition_id_tensor else None
    in_names, out_names, out_avals, zero_outs = [], [], [], []
    in_shapes = []
    for alloc in nc.m.functions[0].allocations:
        if not isinstance(alloc, mybir.MemoryLocationSet):
            continue
        name = alloc.memorylocations[0].name
        if alloc.kind == "ExternalInput":
            if name != partition_name and (nc.dbg_addr is None or name != nc.dbg_addr.name):
                in_names.append(name)
                in_shapes.append((tuple(alloc.tensor_shape), mybir.dt.np(alloc.dtype)))
        elif alloc.kind == "ExternalOutput":
            shape = tuple(alloc.tensor_shape)
            dt = mybir.dt.np(alloc.dtype)
            out_names.append(name)
            out_avals.append(jax.core.ShapedArray(shape, dt))
            zero_outs.append(np.zeros(shape, dt))
    n_params = len(in_names)
    n_outs = len(out_names)
    all_in_names = list(in_names) + list(out_names)
    if nc.dbg_addr is not None:
        all_in_names.append(nc.dbg_addr.name)
    if partition_name is not None:
        all_in_names.append(partition_name)

    def _body(*args):
        operands = list(args)
        if nc.dbg_addr is not None:
            operands.append(jax.numpy.zeros((1, 2), jax.numpy.uint32))
        if partition_name is not None:
            operands.append(partition_id_tensor())
        outs = _bass_exec_p.bind(
            *operands,
            out_avals=tuple(out_avals),
            in_names=tuple(all_in_names),
            out_names=tuple(out_names),
            lowering_input_output_aliases=(),
            sim_require_finite=True,
            sim_require_nnan=True,
            nc=nc,
        )
        return tuple(outs)

    devices = jax.devices()[:n_cores]
    mesh = Mesh(np.asarray(devices), ("core",))
    in_specs = (PartitionSpec("core"),) * (n_params + n_outs)
    out_specs = (PartitionSpec("core"),) * n_outs
    sharded = jax.jit(
        shard_map(_body, mesh=mesh, in_specs=in_specs, out_specs=out_specs,
                  check_rep=False),
        keep_unused=True)
    zeros_concat = [np.zeros((n_cores * z.shape[0], *z.shape[1:]), z.dtype)
                    for z in zero_outs]

    from jax.sharding import NamedSharding
    shard = NamedSharding(mesh, PartitionSpec("core"))
    zeros_dev = jax.device_put(zeros_concat, [shard] * len(zeros_concat)) if zeros_concat else []

    in_avals = [jax.ShapeDtypeStruct((n_cores * s[0], *s[1:]), dt, sharding=shard)
                for s, dt in in_shapes]
    out_zero_avals = [jax.ShapeDtypeStruct(z.shape, z.dtype, sharding=shard)
                      for z in zeros_concat]
    compiled = sharded.lower(*in_avals, *out_zero_avals).compile()

    def fn(concat_inputs):
        return compiled(*concat_inputs, *zeros_dev)

    def put(concat_inputs):
        return jax.device_put(concat_inputs, [shard] * len(concat_inputs))

    return fn, in_names, out_names, put, compiled


_CACHE = {}
_LAST = {}


def _get_compiled(plan_key, plan):
    if plan_key not in _CACHE:
        nc = build_program(plan)
        fn, in_names, out_names, put, compiled = build_runner(nc, plan["n_cores"])
        _CACHE[plan_key] = (nc, fn, in_names, out_names, put, compiled)
    return _CACHE[plan_key]


def run_gat(features, weights_kw, src, dst, n_cores=8, n_timing=0):
    n_nodes = features.shape[0]
    plan = host_preprocess(src, dst, n_nodes, n_cores=n_cores, n_chunks=4)
    weights = host_weights(**weights_kw)
    key = (n_nodes, n_cores, bytes(np.asarray(src[:64]).tobytes()),
           plan["n_tiles"])
    nc, fn, in_names, out_names, put, compiled = _get_compiled(key, plan)
    in_maps = make_in_maps(plan, weights, features)
    concat_in = [np.concatenate([np.asarray(in_maps[c][nm])
                                 for c in range(n_cores)], axis=0)
                 for nm in in_names]
    concat_in = put(concat_in)
    _LAST.update(nc=nc, fn=fn, concat_in=concat_in, plan=plan,
                 compiled=compiled, in_names=in_names, out_names=out_names)
    out = fn(concat_in)
    jax.block_until_ready(out)
    times = []
    if n_timing:
        import time
        for _ in range(n_timing):
            t0 = time.perf_counter()
            out = fn(concat_in)
            jax.block_until_ready(out)
            times.append(time.perf_counter() - t0)
    oi = out_names.index("out")
    arr = np.asarray(out[oi])
    NLP = arr.shape[0] // n_cores
    results = [{"out": arr[c * NLP:(c + 1) * NLP]} for c in range(n_cores)]
    full = assemble_output(plan, results)[:n_nodes]
    return full, times


def kernel(features, W1, al1, ar1, b1, W2, al2, ar2, b2, W3, al3, ar3, b3,
           src, dst):
    wk = dict(W1=W1, al1=al1, ar1=ar1, b1=b1, W2=W2, al2=al2, ar2=ar2, b2=b2,
              W3=W3, al3=al3, ar3=ar3, b3=b3)
    out, _ = run_gat(np.asarray(features, np.float32), wk,
                     np.asarray(src), np.asarray(dst), n_cores=8)
    return out.astype(np.float32)


# revision 41
# speedup vs baseline: 13.8016x; 1.2559x over previous
"""GAT Trainium kernel v3: 256B gather rows + PSUM-decontended DVE ops.

v3 changes vs v2 (driven by NTFF profile: GpSimd 86% busy, 3.65ms in
DMAGatherAnt at ~7.8ns/idx while SDMA queues were only ~20% busy):
- The Q7 descriptor generator shares an SBUF port pair with DVE; every
  2-input DVE op (is_equal, multiply, leaky) locked GpSimd out for its whole
  duration. All hot DVE ops are now 1-SBUF-port: one operand lives in PSUM
  (iota for gen_S, exp/scores in PSUM), leaky-relu + exp moved to the ACT
  engine (Lrelu/Exp), memset of the accumulator moved to ACT.
- G rows shrunk 512B -> 256B (128 bf16 = h only). el is no longer carried in
  the row; the consumer computes el = sum(h*al) per head with a DVE mult +
  tensor_reduce per batch. Halves gather bytes and AllGather wire volume.
- Layer-3 rows stay 256B: [h x40 | el x1 | pad] bf16.
"""
from contextlib import ExitStack
import numpy as np
import ml_dtypes
import concourse.bass as bass
import concourse.tile as tile
from concourse import bacc, mybir
from concourse.masks import make_identity

F32 = mybir.dt.float32
BF16 = mybir.dt.bfloat16
I16 = mybir.dt.int16
BF = ml_dtypes.bfloat16

IN, HID, HEADS, OUT = 256, 32, 4, 40
HH = HID * HEADS  # 128
NEG_SLOPE = 0.2
BT = 16  # tiles per gather batch (16*128 = 2048 idx/call)


def wrap16(a):
    n = a.shape[0]
    assert n % 16 == 0
    blk = a.reshape(-1, 16).T  # [16, n/16]
    return np.tile(blk, (8, 1)).astype(np.int16)


def _balance_positions(indeg, NB, NCH, QR, NL):
    """Assign NL local nodes to positions, keeping each node inside its
    original quarter (so src-chunk assignments stay fixed) while packing
    nodes into 128-row blocks to minimize the max per-(block, chunk)
    in-edge count (the SPMD capacity every core pads to)."""
    pos = np.empty(NL, np.int64)
    load = np.zeros((NB, NCH), np.float64)
    tot = np.zeros(NB, np.float64)
    quarter = np.arange(NL) // QR
    for v in range(NCH):
        p0, p1 = v * QR, (v + 1) * QR
        nodes = np.nonzero(quarter == v)[0]
        b_lo, b_hi = p0 // 128, (p1 - 1) // 128
        bins = np.arange(b_lo, b_hi + 1)
        slots = (np.minimum((bins + 1) * 128, p1)
                 - np.maximum(bins * 128, p0))
        nxt = np.maximum(bins * 128, p0).copy()
        order = nodes[np.argsort(-indeg[nodes].sum(1), kind="stable")]
        for n in order:
            w = indeg[n]
            cand = (load[bins] + w).max(1) + 1e-3 * tot[bins]
            cand[slots == 0] = 1e18
            j = int(cand.argmin())
            b = int(bins[j])
            pos[n] = nxt[j]
            nxt[j] += 1
            slots[j] -= 1
            load[b] += w
            tot[b] += w.sum()
    return pos


def host_preprocess(src, dst, n_nodes, n_cores=8, n_chunks=4):
    NL = n_nodes // n_cores
    assert NL * n_cores == n_nodes
    NB = (NL + 127) // 128
    CS = (n_nodes + n_chunks - 1) // n_chunks
    assert CS <= 32767
    QR = NL // n_chunks
    assert QR * n_chunks == NL
    cap_last = NL - 128 * (NB - 1)

    src = np.asarray(src, np.int64); dst = np.asarray(dst, np.int64)
    core_of = dst // NL

    # --- per-core node->position balancing ---
    # position determines BOTH the node's dst block (pos//128) and its src
    # chunk (pos//QR). Nodes stay inside their original quarter (chunk map
    # fixed, so no cross-core fixed-point issues) while block packing
    # minimizes the max-over-cores (block, chunk) cell count.
    _ = cap_last
    chunk_map = (np.arange(n_nodes) % NL) // QR
    pos_of = np.empty(n_nodes, np.int64)
    for c in range(n_cores):
        m = core_of == c
        dl = dst[m] - c * NL
        ch = chunk_map[src[m]]
        indeg = np.zeros((NL, n_chunks), np.int64)
        np.add.at(indeg, (dl, ch), 1)
        pos_of[c * NL:(c + 1) * NL] = _balance_positions(
            indeg, NB, n_chunks, QR, NL)

    # --- per-core edge lists in (chunk, block) order ---
    per_core = []
    counts = np.zeros((n_cores, NB, n_chunks), np.int64)
    for c in range(n_cores):
        m = core_of == c
        s, d = src[m], dst[m]
        dpos = pos_of[d]            # local position on this core
        b = dpos // 128
        dmod = dpos % 128
        cs_ = s // NL
        spos = pos_of[s]            # local position on src's owner core
        ch = spos // QR
        sloc = cs_ * QR + (spos % QR)   # row within the chunk's gather table
        order = np.lexsort((dmod, b, ch))
        per_core.append((sloc[order], b[order], dmod[order], ch[order]))
        np.add.at(counts, (np.full(b.shape, c), b, ch), 1)
    cap = counts.max(axis=0)  # [NB, n_chunks]

    # --- straddle-packed segments: capacity cap[b,q], packed back to back
    # within each chunk; tiles of 128 may straddle segment boundaries ---
    segs = []          # (chunk, block, g0, capn) in global slot coords
    chunk_tiles = []
    g = 0
    tile_base = []
    for q in range(n_chunks):
        cbase = g
        for b in range(NB):
            n = int(cap[b, q])
            if n > 0:
                segs.append((q, b, g, n))
            g += n
        g = ((g + 127) // 128) * 128
        chunk_tiles.append((g - cbase) // 128)
    total_slots = g
    n_tiles = total_slots // 128

    last_seg_of_block = {}
    for si, (q, b, g0, n) in enumerate(segs):
        last_seg_of_block[b] = si

    # pieces per tile: (cls, seg_idx, first, last). cls 0 ("A") = the piece
    # containing the tile's lane 0 (seg started at/before the tile
    # boundary); cls 1 ("B") = a segment starting mid-tile. Matmuls stay
    # full-tile (tile_position (0,0)) using per-class masked one-hots.
    tile_pieces = [[] for _ in range(n_tiles)]
    for si, (q, b, g0, n) in enumerate(segs):
        g1 = g0 + n
        t0, t1 = g0 // 128, (g1 - 1) // 128
        for t in range(t0, t1 + 1):
            e0 = max(g0, t * 128) - t * 128
            e1 = min(g1, (t + 1) * 128) - t * 128
            cls = 0 if e0 == 0 else 1
            first = t * 128 + e0 == g0
            last = t * 128 + e1 == g1
            tile_pieces[t].append((cls, si, first, last))
    assert all(len(p) <= 2 for p in tile_pieces)

    batches = []
    t = 0
    for q in range(n_chunks):
        nt_q = chunk_tiles[q]
        done = 0
        while done < nt_q:
            nt = min(BT, nt_q - done)
            batches.append({"chunk": q, "t0": t, "nt": nt})
            t += nt
            done += nt
    assert t == n_tiles

    core_data = []
    seg_arr = segs
    for c in range(n_cores):
        sloc, b_arr, dmod_arr, ch_arr = per_core[c]
        src16 = np.zeros(total_slots, np.int16)
        # class-A/B masked one-hot sources; pads (-1) never match
        dcolA = np.full((128, n_tiles), -1.0, np.float32)
        dcolB = np.full((128, n_tiles), -1.0, np.float32)
        S2A = np.zeros((128, total_slots), BF)
        S2B = np.zeros((128, total_slots), BF)
        # edges sorted by (ch, b); segments sorted the same way -> walk
        ptr = 0
        for q, bb, g0, capn in seg_arr:
            n = 0
            while ptr + n < len(b_arr) and ch_arr[ptr + n] == q and b_arr[ptr + n] == bb:
                n += 1
            assert n <= capn
            if n:
                sl = sloc[ptr:ptr + n]
                dm = dmod_arr[ptr:ptr + n]
                slot = g0 + np.arange(n)
                src16[slot] = sl
                tt = slot // 128
                isA = g0 <= tt * 128  # seg contains the tile's lane 0
                la = slot[isA]; lb = slot[~isA]
                dcolA[la % 128, la // 128] = dm[isA]
                dcolB[lb % 128, lb // 128] = dm[~isA]
                S2A[dm[isA], la] = 1.0
                S2B[dm[~isA], lb] = 1.0
            ptr += n
        assert ptr == len(b_arr)
        core_data.append(dict(
            src16=wrap16(src16),
            dcolA=dcolA, dcolB=dcolB,
            S2A=S2A, S2B=S2B,
            pos=np.argsort(pos_of[c * NL:(c + 1) * NL], kind="stable"),
            pos_of=pos_of[c * NL:(c + 1) * NL].copy(),
        ))

    return dict(
        n_cores=n_cores, n_nodes=n_nodes, NL=NL, NB=NB, CS=CS, QR=QR,
        n_chunks=n_chunks, segs=segs, batches=batches, n_tiles=n_tiles,
        tile_pieces=tile_pieces, last_seg_of_block=last_seg_of_block,
        total_slots=total_slots, core_data=core_data,
    )


def host_weights(W1, al1, ar1, b1, W2, al2, ar2, b2, W3, al3, ar3, b3):
    def bd(al):
        al = np.asarray(al, np.float32)
        H, F = al.shape
        out = np.zeros((H * F, H), np.float32)
        for h in range(H):
            out[h * F:(h + 1) * F, h] = al[h]
        return out
    W1 = np.asarray(W1, np.float32); W2 = np.asarray(W2, np.float32); W3 = np.asarray(W3, np.float32)
    # producer rows carry only h (+er table); el computed on the consumer
    W1ext = np.concatenate([W1, W1 @ bd(ar1)], axis=1)            # [IN, 132]
    W2ext = np.concatenate([W2, W2 @ bd(ar2)], axis=1)            # [HH, 132]
    W3ext = np.concatenate([W3, W3 @ bd(al3), W3 @ bd(ar3)], axis=1)  # [HH, 42]
    b1rep = np.tile(np.asarray(b1, np.float32).reshape(1, HH), (128, 1))
    b2rep = np.tile(np.asarray(b2, np.float32).reshape(1, HH), (128, 1))
    b3rep = np.tile(np.asarray(b3, np.float32).reshape(1, OUT), (128, 1))
    iota = np.tile(np.arange(128, dtype=np.float32)[None, :], (128, 1))
    al1rep = np.tile(np.asarray(al1, np.float32).reshape(1, HH), (128, 1))
    al2rep = np.tile(np.asarray(al2, np.float32).reshape(1, HH), (128, 1))
    return dict(W1ext=W1ext.astype(BF), W2ext=W2ext.astype(BF),
                W3ext=W3ext.astype(BF),
                b1rep=b1rep, b2rep=b2rep, b3rep=b3rep,
                iota=iota,
                al1rep=al1rep.astype(BF), al2rep=al2rep.astype(BF))


def apx(base_ap, col_off, dims):
    """AP at column offset of a [128, W] tile with custom free dims."""
    b = base_ap[:, col_off:col_off + 1]
    return bass.AP(b.tensor, b.offset, [b.ap[0]] + [list(d) for d in dims])


def build_program(plan, stage=99):
    C = plan["n_cores"]; NL = plan["NL"]; NB = plan["NB"]
    NT = plan["n_tiles"]; TS = plan["total_slots"]
    NLP = NB * 128

    nc = bacc.Bacc("TRN2", target_bir_lowering=False, debug=False, num_devices=C)

    featT = nc.dram_tensor("featT", [IN, NL], BF16, kind="ExternalInput").ap()
    W1e = nc.dram_tensor("W1ext", [IN, 132], BF16, kind="ExternalInput").ap()
    W2e = nc.dram_tensor("W2ext", [HH, 132], BF16, kind="ExternalInput").ap()
    W3e = nc.dram_tensor("W3ext", [HH, 42], BF16, kind="ExternalInput").ap()
    B1 = nc.dram_tensor("b1rep", [128, HH], F32, kind="ExternalInput").ap()
    B2 = nc.dram_tensor("b2rep", [128, HH], F32, kind="ExternalInput").ap()
    B3 = nc.dram_tensor("b3rep", [128, OUT], F32, kind="ExternalInput").ap()
    AL1 = nc.dram_tensor("al1rep", [128, HH], BF16, kind="ExternalInput").ap()
    AL2 = nc.dram_tensor("al2rep", [128, HH], BF16, kind="ExternalInput").ap()
    SRC = nc.dram_tensor("src16", [128, TS // 16], I16, kind="ExternalInput").ap()
    DCOLA = nc.dram_tensor("dcolA", [128, NT], F32, kind="ExternalInput").ap()
    DCOLB = nc.dram_tensor("dcolB", [128, NT], F32, kind="ExternalInput").ap()
    IOTA = nc.dram_tensor("iota", [128, 128], F32, kind="ExternalInput").ap()
    S2DA = nc.dram_tensor("S2A", [128, TS], BF16, kind="ExternalInput").ap()
    S2DB = nc.dram_tensor("S2B", [128, TS], BF16, kind="ExternalInput").ap()
    OUTT = nc.dram_tensor("out", [NLP, OUT], F32, kind="ExternalOutput").ap()

    G1s = nc.dram_tensor("G1slab", [NL, 128], BF16).ap()
    G2s = nc.dram_tensor("G2slab", [NL, 128], BF16).ap()
    G3s = nc.dram_tensor("G3slab", [NL, 128], BF16).ap()
    G1 = nc.dram_tensor("G1", [C * NL, 128], BF16, addr_space="Shared").ap()
    G2 = nc.dram_tensor("G2", [C * NL, 128], BF16, addr_space="Shared").ap()
    G3 = nc.dram_tensor("G3", [C * NL, 128], BF16, addr_space="Shared").ap()

    rg = [list(range(C))]

    QR = plan["QR"]; NQ = plan["n_chunks"]; CSz = plan["CS"]

    def allgather(slab, full):
        # quarter-interleaved: AG of slab rows [q*QR,(q+1)*QR) from all cores
        # completes gather-chunk q of `full` -> edge phase chunk q can start
        # while later quarters are still gathering
        for q in range(NQ):
            nc.gpsimd.collective_compute(
                "AllGather", mybir.AluOpType.bypass,
                replica_groups=rg,
                ins=[slab[q * QR:(q + 1) * QR, :]],
                outs=[full[q * CSz:(q + 1) * CSz, :]])

    with tile.TileContext(nc) as tc, ExitStack() as ctx:
        const = ctx.enter_context(tc.tile_pool(name="const", bufs=1))
        accp = ctx.enter_context(tc.tile_pool(name="acc", bufs=1))
        constp = ctx.enter_context(tc.tile_pool(name="constp", bufs=1, space="PSUM"))

        w1sb = const.tile([128, 2 * 132], BF16)
        nc.sync.dma_start(w1sb[:, 0:132], W1e[0:128, :])
        nc.sync.dma_start(w1sb[:, 132:264], W1e[128:256, :])
        w2sb = const.tile([128, 132], BF16)
        nc.sync.dma_start(w2sb[:], W2e[:, :])
        w3sb = const.tile([128, 42], BF16)
        nc.sync.dma_start(w3sb[:], W3e[:, :])
        b1sb = const.tile([128, HH], F32)
        nc.sync.dma_start(b1sb[:], B1[:, :])
        b2sb = const.tile([128, HH], F32)
        nc.sync.dma_start(b2sb[:], B2[:, :])
        b3sb = const.tile([128, OUT], F32)
        nc.sync.dma_start(b3sb[:], B3[:, :])
        al1sb = const.tile([128, HH], BF16)
        nc.sync.dma_start(al1sb[:], AL1[:, :])
        al2sb = const.tile([128, HH], BF16)
        nc.sync.dma_start(al2sb[:], AL2[:, :])
        ident = const.tile([128, 128], F32)
        make_identity(nc, ident[:])
        iotasb = const.tile([128, 128], F32)
        nc.sync.dma_start(iotasb[:], IOTA[:, :])
        dcolAsb = const.tile([128, NT], F32)
        nc.sync.dma_start(dcolAsb[:], DCOLA[:, :])
        dcolBsb = const.tile([128, NT], F32)
        nc.sync.dma_start(dcolBsb[:], DCOLB[:, :])
        dcolsb = (dcolAsb, dcolBsb)
        S2D = (S2DA, S2DB)
        srcsb = const.tile([128, TS // 16], I16)
        nc.sync.dma_start(srcsb[:], SRC[:, :])
        # iota parked in PSUM so gen_S is a 1-SBUF-port DVE op
        iotaP = constp.tile([128, 128], F32)
        nc.vector.tensor_copy(iotaP[:], iotasb[:])
        # SBUF er tables, one per layer: [128 (node-in-block), NB*heads]
        er1sb = const.tile([128, NB * 4], BF16)
        er2sb = const.tile([128, NB * 4], BF16)
        er3sb = const.tile([128, NB], BF16)

        def write_node_rows(lp, ps, nr, b, r0, r1, Gn_s, ersb, n_h, n_el):
            """ps [nr, n_h+n_el+n_er] f32 PSUM -> G slab row bf16 + er table."""
            gsb = lp.tile([128, 128], BF16, tag="gsb")
            if n_el:  # layer-3 rows carry el packed after h
                nc.vector.tensor_copy(gsb[:nr, 0:n_h + n_el], ps[:nr, 0:n_h + n_el])
            else:
                nc.vector.tensor_copy(gsb[:nr, 0:n_h], ps[:nr, 0:n_h])
            nc.sync.dma_start(Gn_s[r0:r1, 0:n_h + n_el], gsb[:nr, 0:n_h + n_el])
            ner = ersb.shape[1] // NB
            nc.vector.tensor_copy(ersb[:nr, b * ner:(b + 1) * ner],
                                  ps[:nr, n_h + n_el:n_h + n_el + ner])

        # Layer 1 node phase
        with tc.tile_pool(name="l1n", bufs=3) as lp, \
             tc.tile_pool(name="l1np", bufs=2, space="PSUM") as pp:
            for b in range(NB):
                r0 = b * 128
                r1 = min(r0 + 128, NL)
                nr = r1 - r0
                xt = lp.tile([128, 256], BF16, tag="xt")
                nc.sync.dma_start(xt[:, 0:nr], featT[0:128, r0:r1])
                nc.sync.dma_start(xt[:, 128:128 + nr], featT[128:256, r0:r1])
                ps = pp.tile([128, 132], F32, tag="ps")
                nc.tensor.matmul(ps[:nr, :], xt[:, 0:nr], w1sb[:, 0:132],
                                 start=True, stop=False)
                nc.tensor.matmul(ps[:nr, :], xt[:, 128:128 + nr], w1sb[:, 132:264],
                                 start=False, stop=True)
                write_node_rows(lp, ps, nr, b, r0, r1, G1s, er1sb, 128, 0)

        if stage >= 2:
            allgather(G1s, G1)
        if stage >= 3:
            edge_layer(tc, plan, 1, G1, er1sb, srcsb, dcolsb, iotaP, S2D, accp,
                       w_next=w2sb, b_rep=b1sb, Gn_s=G2s, ersb_n=er2sb,
                       ident=ident, write_node=write_node_rows, alsb=al1sb,
                       node_phase=(stage >= 4))
        if stage >= 5:
            allgather(G2s, G2)
            edge_layer(tc, plan, 2, G2, er2sb, srcsb, dcolsb, iotaP, S2D, accp,
                       w_next=w3sb, b_rep=b2sb, Gn_s=G3s, ersb_n=er3sb,
                       ident=ident, write_node=write_node_rows, alsb=al2sb)
            allgather(G3s, G3)
        if stage >= 6:
            edge_layer3(tc, plan, G3, er3sb, srcsb, dcolsb, iotaP, S2D, accp,
                        b3sb, ident, OUTT)

    nc.compile()
    return nc





def gen_S(nc, Ssb, dcolsb, iotaP, t0, nt):
    """S[e, k, d] = (dcol[e, t0+k] == iota[d]), bf16.

    dcol read from SBUF, iota from PSUM -> single SBUF read port, so the op
    does not lock GpSimd out of the shared port pair during desc-gen.
    """
    Sv = apx(Ssb[:], 0, [[128, nt], [1, 128]])
    dc = apx(dcolsb[:], t0, [[1, nt], [0, 128]])
    io = apx(iotaP[:], 0, [[0, nt], [1, 128]])
    nc.vector.tensor_tensor(out=Sv, in0=dc, in1=io,
                            op=mybir.AluOpType.is_equal)


def edge_layer(tc, plan, lnum, G, ersb, srcsb, dcolsb, iotaP, S2D, accp,
               w_next, b_rep, Gn_s, ersb_n, ident, write_node, alsb,
               node_phase=True):
    nc = tc.nc
    NL = plan["NL"]; NB = plan["NB"]; CS = plan["CS"]
    segs = plan["segs"]; pieces = plan["tile_pieces"]
    last_seg = plan["last_seg_of_block"]

    A = accp.tile([128, NB * 132], F32, tag="A")
    nc.scalar.memzero(A[:])

    NOUT = w_next.shape[1]

    with tc.tile_pool(name=f"e{lnum}", bufs=4) as ep, \
         tc.tile_pool(name=f"e{lnum}b", bufs=3) as bp, \
         tc.tile_pool(name=f"e{lnum}s", bufs=4) as sp, \
         tc.tile_pool(name=f"n{lnum}", bufs=3) as np_, \
         tc.tile_pool(name=f"e{lnum}p", bufs=2, space="PSUM") as pp, \
         tc.tile_pool(name=f"e{lnum}e", bufs=2, space="PSUM") as pe, \
         tc.tile_pool(name=f"n{lnum}p", bufs=1, space="PSUM") as npp:

        def node_block(b):
            if not node_phase:
                return
            r0 = b * 128
            r1 = min(r0 + 128, NL)
            nr = r1 - r0
            Ab = A[:, b * 132:(b + 1) * 132]
            # nodeA bank: [0:128) hv, [128:132) rs
            nodeA = npp.tile([128, 132], F32, tag="nodeA")
            rs = nodeA[:, 128:132]
            hv = nodeA[:, 0:128]
            nc.vector.tensor_scalar_max(rs, Ab[:, 128:132], 1e-30)
            nc.vector.reciprocal(rs, rs)
            hvv = hv.rearrange("p (g f) -> p g f", g=4)
            rsb = apx(nodeA[:], 128, [[1, 4], [0, 32]])
            av = Ab[:, 0:128].rearrange("p (g f) -> p g f", g=4)
            nc.vector.tensor_tensor(out=hvv, in0=av, in1=rsb,
                                    op=mybir.AluOpType.mult)
            hp = np_.tile([128, 128], F32, tag="hp")
            nc.vector.tensor_tensor(out=hp[:], in0=hv, in1=b_rep[:],
                                    op=mybir.AluOpType.add)
            nc.scalar.activation(hp[:], hp[:], mybir.ActivationFunctionType.Relu)
            pst = npp.tile([128, 128], F32, tag="pst")
            nc.tensor.transpose(out=pst[:], in_=hp[:], identity=ident[:])
            hpt = np_.tile([128, 128], BF16, tag="hpt")
            nc.vector.tensor_copy(hpt[:], pst[:])
            ps2 = npp.tile([128, NOUT], F32, tag="ps2")
            nc.tensor.matmul(ps2[:nr, :], hpt[:, 0:nr], w_next[:],
                             start=True, stop=True)
            n_el_next = 1 if NOUT == 42 else 0
            n_h_next = 40 if NOUT == 42 else 128
            write_node(np_, ps2, nr, b, r0, r1, Gn_s, ersb_n, n_h_next, n_el_next)

        ps_cur = {}
        for bt in plan["batches"]:
            nt = bt["nt"]; t0 = bt["t0"]; chh = bt["chunk"]
            nidx = nt * 128
            T = ep.tile([128, BT * 128], BF16, tag="T")
            Tv = T[:].rearrange("p (k d) -> p k d", d=128)[:, 0:nt, :]
            nc.gpsimd.dma_gather(
                Tv, G[chh * CS:(chh + 1) * CS, :],
                srcsb[:, t0 * 8:(t0 + nt) * 8],
                nidx, nidx, 128, single_packet=False)
            # masked S2 one-hot [d, e] streams (A/B piece classes)
            S2Asb = sp.tile([128, BT * 128], BF16, tag="S2A")
            nc.sync.dma_start(S2Asb[:, 0:nt * 128],
                              S2D[0][:, t0 * 128:(t0 + nt) * 128])
            S2Bsb = sp.tile([128, BT * 128], BF16, tag="S2B")
            nc.sync.dma_start(S2Bsb[:, 0:nt * 128],
                              S2D[1][:, t0 * 128:(t0 + nt) * 128])
            # escore bank: [0:4BT) erps, [4BT:8BT) sc, [8BT:12BT) leaky, [12BT:16BT) exp
            esc = pe.tile([128, BT * 16], F32, tag="esc")
            erps = esc[:, 0:BT * 4]
            sc = esc[:, BT * 4:BT * 8]
            scl = esc[:, BT * 8:BT * 12]
            excol = esc[:, BT * 12:BT * 16]
            # er[e, (k,h)] = sum over piece classes of S2cls.T @ erb_block;
            # masked one-hots keep every matmul full-tile
            for k in range(nt):
                pcs = pieces[t0 + k]
                for j, (cls, si, _, _) in enumerate(pcs):
                    _, b, _, _ = segs[si]
                    nc.tensor.matmul(
                        erps[:, k * 4:(k + 1) * 4],
                        (S2Asb if cls == 0 else S2Bsb)[:, k * 128:(k + 1) * 128],
                        ersb[:, b * 4:(b + 1) * 4],
                        start=(j == 0), stop=(j == len(pcs) - 1))
            # el[e, (k,h)] = sum_f T[e,(k,h,f)] * al[(h,f)] : mult + reduce
            eltmp = ep.tile([128, BT * 128], BF16, tag="eltmp")
            etv = eltmp[:].rearrange("p (k d) -> p k d", d=128)[:, 0:nt, :]
            alv = apx(alsb[:], 0, [[0, nt], [1, 128]])
            Tvb = T[:].rearrange("p (k d) -> p k d", d=128)[:, 0:nt, :]
            nc.vector.tensor_tensor(out=etv, in0=Tvb, in1=alv,
                                    op=mybir.AluOpType.mult)
            elsb = np_.tile([128, BT * 4], F32, tag="elsb")
            elv3 = eltmp[:].rearrange("p (k g f) -> p (k g) f", g=4, f=32)[:, 0:nt * 4, :]
            nc.vector.tensor_reduce(elsb[:, 0:nt * 4], elv3,
                                    axis=mybir.AxisListType.X,
                                    op=mybir.AluOpType.add)
            # score = leaky(el + er); el SBUF + er PSUM -> sc PSUM (1-port)
            nc.vector.tensor_tensor(out=sc[:, 0:nt * 4], in0=elsb[:, 0:nt * 4],
                                    in1=erps[:, 0:nt * 4],
                                    op=mybir.AluOpType.add)
            # leaky = max(0.2*sc, sc): two 1-PSUM-read DVE ops (DVE may read
            # only one PSUM input per instruction)
            scm = np_.tile([128, BT * 4], F32, tag="scm")
            nc.vector.tensor_scalar_mul(scm[:, 0:nt * 4], sc[:, 0:nt * 4],
                                        NEG_SLOPE)
            nc.vector.tensor_tensor(out=scl[:, 0:nt * 4], in0=scm[:, 0:nt * 4],
                                    in1=sc[:, 0:nt * 4],
                                    op=mybir.AluOpType.max)
            nc.scalar.activation(excol[:, 0:nt * 4], scl[:, 0:nt * 4],
                                 mybir.ActivationFunctionType.Exp)
            B = bp.tile([128, BT * 132], BF16, tag="B")
            Bv = B[:].rearrange("p (k d) -> p k d", d=132)[:, 0:nt, :]
            # alpha into B's payload cols (for the denominator row of ps)
            nc.vector.tensor_copy(Bv[:, :, 128:132],
                                  excol[:].rearrange("p (k d) -> p k d", d=4)[:, 0:nt, :])
            # weighted h: T (SBUF) * exp (PSUM broadcast) -> B (1 SBUF read)
            hw = apx(B[:], 0, [[132, nt], [32, 4], [1, 32]])
            hi = apx(T[:], 0, [[128, nt], [32, 4], [1, 32]])
            ex4 = apx(esc[:], BT * 12, [[4, nt], [1, 4], [0, 32]])
            nc.vector.tensor_tensor(out=hw, in0=hi, in1=ex4,
                                    op=mybir.AluOpType.mult)
            SAsb = sp.tile([128, BT * 128], BF16, tag="SA")
            gen_S(nc, SAsb, dcolsb[0], iotaP, t0, nt)
            SBsb = sp.tile([128, BT * 128], BF16, tag="SB")
            gen_S(nc, SBsb, dcolsb[1], iotaP, t0, nt)
            for k in range(nt):
                for cls, si, first, last in pieces[t0 + k]:
                    _, b, _, _ = segs[si]
                    if first:
                        ps_cur[si] = pp.tile([128, 132], F32, tag="ps", name="ps")
                    ps = ps_cur[si]
                    nc.tensor.matmul(
                        ps[:],
                        (SAsb if cls == 0 else SBsb)[:, k * 128:(k + 1) * 128],
                        B[:, k * 132:(k + 1) * 132],
                        start=first, stop=last)
                    if last:
                        nc.vector.tensor_tensor(
                            out=A[:, b * 132:(b + 1) * 132],
                            in0=A[:, b * 132:(b + 1) * 132],
                            in1=ps[:], op=mybir.AluOpType.add)
                        del ps_cur[si]
                        if si == last_seg[b]:
                            node_block(b)
        assert not ps_cur
        for b in range(NB):
            if b not in last_seg:
                node_block(b)


def edge_layer3(tc, plan, G, ersb, srcsb, dcolsb, iotaP, S2D, accp,
                b3sb, ident, OUTT):
    nc = tc.nc
    NL = plan["NL"]; NB = plan["NB"]; CS = plan["CS"]
    segs = plan["segs"]; pieces = plan["tile_pieces"]
    last_seg = plan["last_seg_of_block"]

    A = accp.tile([128, NB * 132], F32, tag="A")
    Av = A[:, 0:NB * 41]
    nc.scalar.memzero(A[:])

    with tc.tile_pool(name="n3", bufs=1) as no_, \
         tc.tile_pool(name="n3p", bufs=2, space="PSUM") as nop:
        O = no_.tile([128, NB * 40], F32, tag="O")

        def node_block3(b):
            Ab = Av[:, b * 41:(b + 1) * 41]
            rs = nop.tile([128, 4], F32, tag="rs3")
            nc.vector.tensor_scalar_max(rs[:, 0:1], Ab[:, 40:41], 1e-30)
            nc.vector.reciprocal(rs[:, 0:1], rs[:, 0:1])
            rsb = apx(rs[:], 0, [[0, 40]])
            Ob = O[:, b * 40:(b + 1) * 40]
            nc.vector.tensor_tensor(out=Ob, in0=Ab[:, 0:40], in1=rsb,
                                    op=mybir.AluOpType.mult)
            nc.vector.tensor_tensor(out=Ob, in0=Ob, in1=b3sb[:],
                                    op=mybir.AluOpType.add)

        with tc.tile_pool(name="e3", bufs=4) as ep, \
             tc.tile_pool(name="e3b", bufs=3) as bp, \
             tc.tile_pool(name="e3s", bufs=4) as sp, \
             tc.tile_pool(name="e3p", bufs=3, space="PSUM") as pp, \
             tc.tile_pool(name="e3e", bufs=2, space="PSUM") as pe:
            ps_cur = {}
            for bt in plan["batches"]:
                nt = bt["nt"]; t0 = bt["t0"]; chh = bt["chunk"]
                nidx = nt * 128
                T = ep.tile([128, BT * 128], BF16, tag="T3")
                Tv = T[:].rearrange("p (k d) -> p k d", d=128)[:, 0:nt, :]
                nc.gpsimd.dma_gather(
                    Tv, G[chh * CS:(chh + 1) * CS, :],
                    srcsb[:, t0 * 8:(t0 + nt) * 8],
                    nidx, nidx, 128, single_packet=False)
                S2Asb = sp.tile([128, BT * 128], BF16, tag="S23A")
                nc.sync.dma_start(S2Asb[:, 0:nt * 128],
                                  S2D[0][:, t0 * 128:(t0 + nt) * 128])
                S2Bsb = sp.tile([128, BT * 128], BF16, tag="S23B")
                nc.sync.dma_start(S2Bsb[:, 0:nt * 128],
                                  S2D[1][:, t0 * 128:(t0 + nt) * 128])
                # escore bank: [0:BT) erps, [BT:2BT) sc, [2BT:3BT) leaky, [3BT:4BT) exp
                esc = pe.tile([128, BT * 4], F32, tag="esc3")
                erps = esc[:, 0:BT]
                sc = esc[:, BT:2 * BT]
                scl = esc[:, 2 * BT:3 * BT]
                excol = esc[:, 3 * BT:4 * BT]
                for k in range(nt):
                    pcs = pieces[t0 + k]
                    for j, (cls, si, _, _) in enumerate(pcs):
                        _, b, _, _ = segs[si]
                        nc.tensor.matmul(
                            erps[:, k:k + 1],
                            (S2Asb if cls == 0 else S2Bsb)[:, k * 128:(k + 1) * 128],
                            ersb[:, b:b + 1],
                            start=(j == 0), stop=(j == len(pcs) - 1))
                # el is packed bf16 in the row at col 40
                scv = sc[:].rearrange("p (k d) -> p k d", d=1)[:, 0:nt, :]
                erv = erps[:].rearrange("p (k d) -> p k d", d=1)[:, 0:nt, :]
                nc.vector.tensor_tensor(out=scv, in0=Tv[:, :, 40:41], in1=erv,
                                        op=mybir.AluOpType.add)
                scm = no_.tile([128, BT], F32, tag="scm3", bufs=2)
                nc.vector.tensor_scalar_mul(scm[:, 0:nt], sc[:, 0:nt],
                                            NEG_SLOPE)
                nc.vector.tensor_tensor(out=scl[:, 0:nt], in0=scm[:, 0:nt],
                                        in1=sc[:, 0:nt],
                                        op=mybir.AluOpType.max)
                nc.scalar.activation(excol[:, 0:nt], scl[:, 0:nt],
                                     mybir.ActivationFunctionType.Exp)
                B = bp.tile([128, BT * 41], BF16, tag="B3")
                Bv = B[:].rearrange("p (k d) -> p k d", d=41)[:, 0:nt, :]
                nc.vector.tensor_copy(
                    Bv[:, :, 40:41],
                    excol[:].rearrange("p (k d) -> p k d", d=1)[:, 0:nt, :])
                hw = apx(B[:], 0, [[41, nt], [1, 40]])
                hi = apx(T[:], 0, [[128, nt], [1, 40]])
                ex1 = apx(esc[:], 3 * BT, [[1, nt], [0, 40]])
                nc.vector.tensor_tensor(out=hw, in0=hi, in1=ex1,
                                        op=mybir.AluOpType.mult)
                SAsb = sp.tile([128, BT * 128], BF16, tag="S3A")
                gen_S(nc, SAsb, dcolsb[0], iotaP, t0, nt)
                SBsb = sp.tile([128, BT * 128], BF16, tag="S3B")
                gen_S(nc, SBsb, dcolsb[1], iotaP, t0, nt)
                for k in range(nt):
                    for cls, si, first, last in pieces[t0 + k]:
                        _, b, _, _ = segs[si]
                        if first:
                            ps_cur[si] = pp.tile([128, 41], F32, tag="ps3",
                                                 name="ps3")
                        ps = ps_cur[si]
                        nc.tensor.matmul(
                            ps[:],
                            (SAsb if cls == 0 else SBsb)[:, k * 128:(k + 1) * 128],
                            B[:, k * 41:(k + 1) * 41],
                            start=first, stop=last)
                        if last:
                            nc.vector.tensor_tensor(
                                out=Av[:, b * 41:(b + 1) * 41],
                                in0=Av[:, b * 41:(b + 1) * 41],
                                in1=ps[:], op=mybir.AluOpType.add)
                            del ps_cur[si]
                            if si == last_seg[b]:
                                node_block3(b)
            assert not ps_cur
            for b in range(NB):
                if b not in last_seg:
                    node_block3(b)

        Ovv = O[:].rearrange("p (b f) -> p b f", f=40)
        mx = no_.tile([128, NB], F32, tag="mx")
        nc.vector.tensor_reduce(mx[:], Ovv, axis=mybir.AxisListType.X,
                                op=mybir.AluOpType.max)
        mxb = apx(mx[:], 0, [[1, NB], [0, 40]])
        nc.vector.tensor_tensor(out=Ovv, in0=Ovv, in1=mxb,
                                op=mybir.AluOpType.subtract)
        E = no_.tile([128, NB * 40], F32, tag="E")
        nc.scalar.activation(E[:], O[:], mybir.ActivationFunctionType.Exp)
        ss = no_.tile([128, NB], F32, tag="ss")
        nc.vector.tensor_reduce(ss[:], E[:].rearrange("p (b f) -> p b f", f=40),
                                axis=mybir.AxisListType.X, op=mybir.AluOpType.add)
        nc.scalar.activation(ss[:], ss[:], mybir.ActivationFunctionType.Ln)
        ssb = apx(ss[:], 0, [[1, NB], [0, 40]])
        nc.vector.tensor_tensor(out=Ovv, in0=Ovv, in1=ssb,
                                op=mybir.AluOpType.subtract)
        nc.sync.dma_start(OUTT[:, :].rearrange("(b p) f -> p b f", p=128), Ovv)


def make_in_maps(plan, weights, features):
    """Per-core input dicts."""
    C = plan["n_cores"]; NL = plan["NL"]
    features = np.asarray(features, np.float32).astype(BF)
    maps = []
    for c in range(C):
        cd = plan["core_data"][c]
        maps.append(dict(
            featT=np.ascontiguousarray(features[c * NL:(c + 1) * NL][cd["pos"]].T),
            W1ext=weights["W1ext"], W2ext=weights["W2ext"], W3ext=weights["W3ext"],
            b1rep=weights["b1rep"], b2rep=weights["b2rep"], b3rep=weights["b3rep"],
            al1rep=weights["al1rep"], al2rep=weights["al2rep"],
            iota=weights["iota"],
            src16=cd["src16"], dcolA=cd["dcolA"], dcolB=cd["dcolB"],
            S2A=cd["S2A"], S2B=cd["S2B"],
        ))
    return maps


def assemble_output(plan, results):
    C = plan["n_cores"]; NL = plan["NL"]
    outs = []
    for c in range(C):
        pos_of = plan["core_data"][c]["pos_of"]  # local node -> position
        outs.append(results[c]["out"][pos_of])
    return np.concatenate(outs, axis=0)


# ---------------- execution harness (PJRT via bass2jax) ----------------
import jax
from jax.sharding import Mesh, PartitionSpec
from jax.experimental.shard_map import shard_map
from concourse.bass2jax import _bass_exec_p, partition_id_tensor, install_neuronx_cc_hook


def build_runner(nc, n_cores):
    install_neuronx_cc_hook()
    partition_name = nc.partition_id_tensor.name if nc.partition_id_tensor else None
    in_names, out_names, out_avals, zero_outs = [], [], [], []
    in_shapes = []
    for alloc in nc.m.functions[0].allocations:
        if not isinstance(alloc, mybir.MemoryLocationSet):
            continue
        name = alloc.memorylocations[0].name
        if alloc.kind == "ExternalInput":
            if name != partition_name and (nc.dbg_addr is None or name != nc.dbg_addr.name):
                in_names.append(name)
                in_shapes.append((tuple(alloc.tensor_shape), mybir.dt.np(alloc.dtype)))
        elif alloc.kind == "ExternalOutput":
            shape = tuple(alloc.tensor_shape)
            dt = mybir.dt.np(alloc.dtype)
            out_names.append(name)
            out_avals.append(jax.core.ShapedArray(shape, dt))
            zero_outs.append(np.zeros(shape, dt))
    n_params = len(in_names)
    n_outs = len(out_names)
    all_in_names = list(in_names) + list(out_names)
    if nc.dbg_addr is not None:
        all_in_names.append(nc.dbg_addr.name)
    if partition_name is not None:
        all_in_names.append(partition_name)

    def _body(*args):
        operands = list(args)
        if nc.dbg_addr is not None:
            operands.append(jax.numpy.zeros((1, 2), jax.numpy.uint32))
        if partition_name is not None:
            operands.append(partition_id_tensor())
        outs = _bass_exec_p.bind(
            *operands,
            out_avals=tuple(out_avals),
            in_names=tuple(all_in_names),
            out_names=tuple(out_names),
            lowering_input_output_aliases=(),
            sim_require_finite=True,
            sim_require_nnan=True,
            nc=nc,
        )
        return tuple(outs)

    devices = jax.devices()[:n_cores]
    mesh = Mesh(np.asarray(devices), ("core",))
    in_specs = (PartitionSpec("core"),) * (n_params + n_outs)
    out_specs = (PartitionSpec("core"),) * n_outs
    sharded = jax.jit(
        shard_map(_body, mesh=mesh, in_specs=in_specs, out_specs=out_specs,
                  check_rep=False),
        keep_unused=True)
    zeros_concat = [np.zeros((n_cores * z.shape[0], *z.shape[1:]), z.dtype)
                    for z in zero_outs]

    from jax.sharding import NamedSharding
    shard = NamedSharding(mesh, PartitionSpec("core"))
    zeros_dev = jax.device_put(zeros_concat, [shard] * len(zeros_concat)) if zeros_concat else []

    in_avals = [jax.ShapeDtypeStruct((n_cores * s[0], *s[1:]), dt, sharding=shard)
                for s, dt in in_shapes]
    out_zero_avals = [jax.ShapeDtypeStruct(z.shape, z.dtype, sharding=shard)
                      for z in zeros_concat]
    compiled = sharded.lower(*in_avals, *out_zero_avals).compile()

    def fn(concat_inputs):
        return compiled(*concat_inputs, *zeros_dev)

    def put(concat_inputs):
        return jax.device_put(concat_inputs, [shard] * len(concat_inputs))

    return fn, in_names, out_names, put, compiled


_CACHE = {}
_LAST = {}


def _get_compiled(plan_key, plan):
    if plan_key not in _CACHE:
        nc = build_program(plan)
        fn, in_names, out_names, put, compiled = build_runner(nc, plan["n_cores"])
        _CACHE[plan_key] = (nc, fn, in_names, out_names, put, compiled)
    return _CACHE[plan_key]


def run_gat(features, weights_kw, src, dst, n_cores=8, n_timing=0):
    n_nodes = features.shape[0]
    plan = host_preprocess(src, dst, n_nodes, n_cores=n_cores, n_chunks=4)
    weights = host_weights(**weights_kw)
    key = (n_nodes, n_cores, bytes(np.asarray(src[:64]).tobytes()),
           plan["n_tiles"])
    nc, fn, in_names, out_names, put, compiled = _get_compiled(key, plan)
    in_maps = make_in_maps(plan, weights, features)
    concat_in = [np.concatenate([np.asarray(in_maps[c][nm])
                                 for c in range(n_cores)], axis=0)
                 for nm in in_names]
    concat_in = put(concat_in)
    _LAST.update(nc=nc, fn=fn, concat_in=concat_in, plan=plan,
                 compiled=compiled, in_names=in_names, out_names=out_names)
    out = fn(concat_in)
    jax.block_until_ready(out)
    times = []
    if n_timing:
        import time
        for _ in range(n_timing):
            t0 = time.perf_counter()
            out = fn(concat_in)
            jax.block_until_ready(out)
            times.append(time.perf_counter() - t0)
    oi = out_names.index("out")
    arr = np.asarray(out[oi])
    NLP = arr.shape[0] // n_cores
    results = [{"out": arr[c * NLP:(c + 1) * NLP]} for c in range(n_cores)]
    full = assemble_output(plan, results)[:n_nodes]
    return full, times


def kernel(features, W1, al1, ar1, b1, W2, al2, ar2, b2, W3, al3, ar3, b3,
           src, dst):
    wk = dict(W1=W1, al1=al1, ar1=ar1, b1=b1, W2=W2, al2=al2, ar2=ar2, b2=b2,
              W3=W3, al3=al3, ar3=ar3, b3=b3)
    out, _ = run_gat(np.asarray(features, np.float32), wk,
                     np.asarray(src), np.asarray(dst), n_cores=8)
    return out.astype(np.float32)
